# revision 11
# baseline (speedup 1.0000x reference)
"""Trainium2 Bass kernel for nn_Block_6433861009667 (transformer block w/ MoE MLP).

Sharding: 8 cores = (batch sample b = core//2) x (token half = core%2).
Each core computes its 512 tokens fully locally (K/V computed for the whole
sample on both cores of a pair -> no collectives at all).

Device layout: feature-major activations [C(partitions), T(free)], fp32r
matmuls (full PE rate, ~1.4e-4 matmul error), softmax denominators computed
on the PE via packed M=1 ones-matmuls, LayerNorm scale/shift folded into the
following matmul weights on the host.
"""
import sys

if "/opt/trn_rl_repo" not in sys.path:
    sys.path.insert(0, "/opt/trn_rl_repo")

import numpy as np

import concourse.bass as bass
import concourse.tile as tile
from concourse import bacc, mybir
from concourse.bass import ts

B, N, C = 4, 1024, 1024
H, HD = 16, 64
HID = 4 * C
PF, SHARED = 256, C - 256
EPS = 1e-5
T = 512            # tokens per core
P = 128
KC = C // P        # 8 feature chunks
MC_FC1 = HID // P  # 32
F32R = mybir.dt.float32r
F32 = mybir.dt.float32
AF = mybir.ActivationFunctionType


def build():
    nc = bacc.Bacc(trn_type="TRN2")

    # ---- DRAM I/O (per core) ----
    xt_d = nc.dram_tensor("xt", [C, N], F32R, kind="ExternalInput")          # x[b].T (tokens rolled so own half = cols 0:T)
    wqkv_d = nc.dram_tensor("wqkv", [C, 3 * C], F32R, kind="ExternalInput")  # folded (qkv_w*ln1w).T, Q cols pre-scaled
    qkvb_d = nc.dram_tensor("qkvb", [P, 16], F32R, kind="ExternalInput")     # Q/K bias per m-chunk column
    vbias_d = nc.dram_tensor("vbias", [1, C], F32R, kind="ExternalInput")    # V bias row
    wproj_d = nc.dram_tensor("wproj", [C, C], F32R, kind="ExternalInput")    # proj_w.T
    projb_d = nc.dram_tensor("projb", [P, KC], F32R, kind="ExternalInput")
    wfc1_d = nc.dram_tensor("wfc1", [C, HID], F32R, kind="ExternalInput")    # (fc1_w*ln2w).T
    fc1b_d = nc.dram_tensor("fc1b", [P, MC_FC1], F32R, kind="ExternalInput")
    w2_d = nc.dram_tensor("w2", [HID, C], F32R, kind="ExternalInput")        # concat([fc2_w, exp_w[b]]).T
    b2_d = nc.dram_tensor("b2", [P, KC], F32R, kind="ExternalInput")
    ones_d = nc.dram_tensor("ones", [1, P], F32R, kind="ExternalInput")
    out_d = nc.dram_tensor("out", [C, T], F32, kind="ExternalOutput")        # y[b, half].T

    with tile.TileContext(nc, pool_alloc_mode="queue") as tc:
        # ---- whole-kernel pools ----
        root = tc.alloc_tile_pool(name="root", bufs=1)
        tmp = tc.alloc_tile_pool(name="tmp", bufs=2)
        small = tc.alloc_tile_pool(name="small", bufs=1)
        wpool = tc.alloc_tile_pool(name="w", bufs=3)
        # PSUM budget: 3 + 2 + 2 = 7 banks of 8
        mm_ps = tc.alloc_tile_pool(name="mmps", bufs=3, space="PSUM")   # dense mm + attn S
        b2_ps = tc.alloc_tile_pool(name="b2ps", bufs=2, space="PSUM")   # attn O + LN stats
        b3_ps = tc.alloc_tile_pool(name="b3ps", bufs=2, space="PSUM")   # attn D + replicate

        # ---- constants / biases ----
        ones_sq = root.tile([P, P], F32R, tag="ones_sq")       # all-ones
        nc.sync.dma_start(ones_sq[:], ones_d.ap()[0, :].partition_broadcast(P))
        ones_col = root.tile([P, 1], F32R, tag="ones_col")
        nc.sync.dma_start(ones_col[:], ones_d.ap()[0, 0:1].partition_broadcast(P))
        qkvb_s = root.tile([P, 16], F32R, tag="qkvb")
        nc.sync.dma_start(qkvb_s[:], qkvb_d.ap())
        vbias_s = root.tile([1, C], F32R, tag="vbias")
        nc.sync.dma_start(vbias_s[:], vbias_d.ap())
        projb_s = root.tile([P, KC], F32R, tag="projb")
        nc.sync.dma_start(projb_s[:], projb_d.ap())
        fc1b_s = root.tile([P, MC_FC1], F32R, tag="fc1b")
        nc.sync.dma_start(fc1b_s[:], fc1b_d.ap())
        b2_s = root.tile([P, KC], F32R, tag="b2")
        nc.sync.dma_start(b2_s[:], b2_d.ap())
        eps_t = root.tile([1, 1], F32, tag="eps")
        nc.vector.memset(eps_t[:], EPS)

        # =============== helper: feature-major LayerNorm ===============
        def layernorm_fm(src, dst, tokens, aB, cB):
            """src/dst: [P, KC, tokens] f32r tiles. dst = (src - mu)*rstd per token."""
            halves = tokens // T
            negmu = small.tile([1, N], F32R, tag="lnm", name=f"negmu{tokens}")
            msq = small.tile([1, N], F32R, tag="lns", name=f"msq{tokens}")
            crow = small.tile([1, N], F32R, tag="lnc", name=f"crow{tokens}")
            for h in range(halves):
                sum_ps = b2_ps.tile([1, T], F32, tag="bank2", name=f"sum_ps{tokens}_{h}")
                sq_ps = b2_ps.tile([1, T], F32, tag="bank2", name=f"sq_ps{tokens}_{h}")
                for kc in range(KC):
                    xsq = tmp.tile([P, T], F32R, tag="xsq", name=f"xsq{tokens}_{h}_{kc}")
                    nc.vector.tensor_mul(xsq[:], src[:, kc, ts(h, T)], src[:, kc, ts(h, T)])
                    nc.tensor.matmul(sum_ps[:], ones_col[:], src[:, kc, ts(h, T)],
                                     start=(kc == 0), stop=(kc == KC - 1))
                    nc.tensor.matmul(sq_ps[:], ones_col[:], xsq[:],
                                     start=(kc == 0), stop=(kc == KC - 1))
                nc.scalar.mul(negmu[:, ts(h, T)], sum_ps[:], -1.0 / C)
                nc.scalar.mul(msq[:, ts(h, T)], sq_ps[:], 1.0 / C)
            tk = slice(0, tokens)
            nc.vector.tensor_mul(crow[:, tk], negmu[:, tk], negmu[:, tk])
            nc.vector.tensor_sub(msq[:, tk], msq[:, tk], crow[:, tk])
            # rstd = exp(-0.5*ln(var+eps))  (stays inside the exp/ln ACT table set)
            nc.scalar.activation(msq[:, tk], msq[:, tk], AF.Ln, bias=eps_t[0:1, :])
            nc.scalar.activation(msq[:, tk], msq[:, tk], AF.Exp, scale=-0.5)
            nc.vector.tensor_mul(crow[:, tk], negmu[:, tk], msq[:, tk])
            for h in range(halves):
                ra = b3_ps.tile([P, T], F32, tag="bank3", name=f"ra{tokens}_{h}")
                nc.tensor.matmul(ra[:], ones_sq[0:1, :], msq[:, ts(h, T)], start=True, stop=True)
                nc.scalar.copy(aB[:, ts(h, T)], ra[:])
                rc = b3_ps.tile([P, T], F32, tag="bank3", name=f"rc{tokens}_{h}")
                nc.tensor.matmul(rc[:], ones_sq[0:1, :], crow[:, ts(h, T)], start=True, stop=True)
                nc.scalar.copy(cB[:, ts(h, T)], rc[:])
            for kc in range(KC):
                nc.vector.tensor_mul(dst[:, kc, :], src[:, kc, :], aB[:])
                nc.vector.tensor_add(dst[:, kc, :], dst[:, kc, :], cB[:])

        # =============== LN1 ===============
        pLN1 = tc.alloc_tile_pool(name="pLN1", bufs=1)    # ln1 (until V done)
        ln1 = pLN1.tile([P, KC, N], F32R, tag="ln1")
        pWV = tc.alloc_tile_pool(name="pWV", bufs=1)      # wv (until V done)
        wv_s = pWV.tile([P, KC, C], F32R, tag="wv")
        nc.sync.dma_start(wv_s[:], wqkv_d.ap()[:, 2 * C:3 * C].rearrange("(kc p) m -> p kc m", p=P))

        pXT = tc.alloc_tile_pool(name="pXT", bufs=1)      # xt + LN1 broadcast rows
        xt_s = pXT.tile([P, KC, N], F32R, tag="xt")
        nc.sync.dma_start(xt_s[:], xt_d.ap().rearrange("(kc p) t -> p kc t", p=P))
        aB1 = pXT.tile([P, N], F32R, tag="aB1")
        cB1 = pXT.tile([P, N], F32R, tag="cB1")
        layernorm_fm(xt_s, ln1, N, aB1, cB1)
        pXT.release()

        # =============== QKV ===============
        pQ = tc.alloc_tile_pool(name="pQ", bufs=1, side="right")
        pK = tc.alloc_tile_pool(name="pK", bufs=1, side="right")
        pV = tc.alloc_tile_pool(name="pV", bufs=1, side="right")
        q_s = pQ.tile([P, KC, T], F32R, tag="q")
        k_s = pK.tile([P, KC, N], F32R, tag="k")
        vp = pV.tile([P, KC, C], F32R, tag="vp")

        # Q: out[m, t] for this core's (pre-rolled) token half = cols 0:T
        for m in range(KC):
            wt = wpool.tile([P, KC, P], F32R, tag="w128", name=f"wq{m}")
            nc.sync.dma_start(wt[:], wqkv_d.ap()[:, ts(m, P)].rearrange("(kc p) m -> p kc m", p=P))
            ps = mm_ps.tile([P, T], F32, tag="mm", name=f"psq{m}")
            for kc in range(KC):
                nc.tensor.matmul(ps[:], wt[:, kc, :], ln1[:, kc, 0:T],
                                 start=(kc == 0), stop=(kc == KC - 1))
            nc.scalar.activation(q_s[:, m, :], ps[:], AF.Identity, bias=qkvb_s[:, m:m + 1])
        # K: out[m, all N tokens]
        for m in range(KC):
            wt = wpool.tile([P, KC, P], F32R, tag="w128", name=f"wk{m}")
            nc.sync.dma_start(wt[:], wqkv_d.ap()[:, C + m * P:C + (m + 1) * P].rearrange("(kc p) m -> p kc m", p=P))
            for h in range(2):
                ps = mm_ps.tile([P, T], F32, tag="mm", name=f"psk{m}_{h}")
                for kc in range(KC):
                    nc.tensor.matmul(ps[:], wt[:, kc, :], ln1[:, kc, ts(h, T)],
                                     start=(kc == 0), stop=(kc == KC - 1))
                nc.scalar.activation(k_s[:, m, ts(h, T)], ps[:], AF.Identity,
                                     bias=qkvb_s[:, 8 + m:9 + m])
        # V (token-major): out[j, d] ; lhsT = ln1 chunk (j slice), rhs = wv
        for jc in range(KC):
            for dh in range(2):
                ps = mm_ps.tile([P, T], F32, tag="mm", name=f"psv{jc}_{dh}")
                for kc in range(KC):
                    nc.tensor.matmul(ps[:], ln1[:, kc, ts(jc, P)], wv_s[:, kc, ts(dh, T)],
                                     start=(kc == 0), stop=False)
                nc.tensor.matmul(ps[:], ones_sq[0:1, :], vbias_s[:, ts(dh, T)],
                                 start=False, stop=True)
                nc.scalar.copy(vp[:, jc, ts(dh, T)], ps[:])
        pWV.release()
        pLN1.release()

        # =============== Attention (head pairs) ===============
        pX1 = tc.alloc_tile_pool(name="pX1", bufs=1)      # x1 (until end; below pATT on left stack)
        pATT = tc.alloc_tile_pool(name="pATT", bufs=1)    # attn (until proj done)
        attn = pATT.tile([P, KC, T], F32R, tag="attn")
        pPH = tc.alloc_tile_pool(name="pPH", bufs=8, side="right")      # per-(head,jc) P^T tiles

        for c in range(KC):
            phs = [[None] * KC, [None] * KC]
            for jc in range(KC):
                for g in range(2):
                    b0 = g * 64
                    ps = mm_ps.tile([P, T], F32, tag="mm", name=f"s{c}_{g}_{jc}")
                    nc.tensor.matmul(ps[:], k_s[b0:b0 + 64, c, ts(jc, P)],
                                     q_s[b0:b0 + 64, c, :], start=True, stop=True)
                    ph = pPH.tile([P, T], F32R, tag="ph", name=f"ph{c}_{g}_{jc}")
                    nc.scalar.activation(ph[:], ps[:], AF.Exp)
                    phs[g][jc] = ph
            for g in range(2):
                h = 2 * c + g
                pso = b2_ps.tile([64, T], F32, tag="bank2", name=f"pso{c}_{g}")
                psd = b3_ps.tile([1, T], F32, tag="bank3", name=f"psd{c}_{g}")
                for jc in range(KC):
                    st = (jc == 0)
                    sp = (jc == KC - 1)
                    nc.tensor.matmul(pso[:], vp[:, jc, h * 64:(h + 1) * 64],
                                     phs[g][jc][:], start=st, stop=sp)
                    nc.tensor.matmul(psd[:], ones_col[:], phs[g][jc][:],
                                     start=st, stop=sp)
                rcp = tmp.tile([1, T], F32R, tag="rcp", name=f"rcp{c}_{g}")
                nc.scalar.copy(rcp[:], psd[:])
                with nc.allow_low_precision(reason="f32r bits are full fp32 here"):
                    nc.vector.reciprocal(rcp[:], rcp[:])
                psr = b3_ps.tile([64, T], F32, tag="bank3", name=f"psr{c}_{g}")
                nc.tensor.matmul(psr[:], ones_sq[0:1, 0:64], rcp[0:1, :],
                                 start=True, stop=True)
                if g == 0:
                    nc.scalar.copy(attn[0:64, c, :], pso[:])
                    nc.vector.tensor_mul(attn[0:64, c, :], attn[0:64, c, :], psr[:])
                else:
                    ot = tmp.tile([64, T], F32R, tag="ot", name=f"ot{c}")
                    nc.scalar.copy(ot[:], pso[:])
                    nc.vector.tensor_mul(ot[:], ot[:], psr[:])
                    nc.sync.dma_start(attn[64:128, c, :], ot[:])
        pPH.release()
        pV.release()
        pK.release()
        pQ.release()

        # =============== proj + residual ===============
        x1 = pX1.tile([P, KC, T], F32R, tag="x1")
        for m in range(KC):
            wt = wpool.tile([P, KC, P], F32R, tag="w128", name=f"wp{m}")
            nc.sync.dma_start(wt[:], wproj_d.ap()[:, ts(m, P)].rearrange("(kc p) m -> p kc m", p=P))
            xh = tmp.tile([P, T], F32R, tag="xh", name=f"xh{m}")
            nc.sync.dma_start(xh[:], xt_d.ap()[ts(m, P), 0:T])
            ps = mm_ps.tile([P, T], F32, tag="mm", name=f"psp{m}")
            for kc in range(KC):
                nc.tensor.matmul(ps[:], wt[:, kc, :], attn[:, kc, :],
                                 start=(kc == 0), stop=(kc == KC - 1))
            po = tmp.tile([P, T], F32R, tag="po", name=f"po{m}")
            nc.scalar.activation(po[:], ps[:], AF.Identity, bias=projb_s[:, m:m + 1])
            nc.vector.tensor_add(x1[:, m, :], po[:], xh[:])
        pATT.release()

        # =============== LN2 ===============
        pLN2 = tc.alloc_tile_pool(name="pLN2", bufs=1, side="right")
        ln2t = pLN2.tile([P, KC, T], F32R, tag="ln2t")
        aB2 = pLN2.tile([P, T], F32R, tag="aB2")
        cB2 = pLN2.tile([P, T], F32R, tag="cB2")
        layernorm_fm(x1, ln2t, T, aB2, cB2)

        # =============== FC1 + gelu ===============
        pH = tc.alloc_tile_pool(name="pH", bufs=1)
        hbuf = pH.tile([P, MC_FC1, T], F32R, tag="h")
        for m in range(MC_FC1):
            wt = wpool.tile([P, KC, P], F32R, tag="w128", name=f"wf{m}")
            nc.sync.dma_start(wt[:], wfc1_d.ap()[:, ts(m, P)].rearrange("(kc p) m -> p kc m", p=P))
            ps = mm_ps.tile([P, T], F32, tag="mm", name=f"psf{m}")
            for kc in range(KC):
                nc.tensor.matmul(ps[:], wt[:, kc, :], ln2t[:, kc, :],
                                 start=(kc == 0), stop=(kc == KC - 1))
            nc.scalar.activation(hbuf[:, m, :], ps[:], AF.Gelu, bias=fc1b_s[:, m:m + 1])
        pLN2.release()

        # =============== FC2 (shared+expert fused) + residual + out ===============
        pW2 = tc.alloc_tile_pool(name="pW2", bufs=2, side="right")
        pY = tc.alloc_tile_pool(name="pY", bufs=1, side="right")
        ybuf = pY.tile([P, KC, T], F32, tag="y")
        for m in range(KC):
            wt = pW2.tile([P, MC_FC1, P], F32R, tag="w2t", name=f"w2_{m}")
            nc.sync.dma_start(wt[:], w2_d.ap()[:, ts(m, P)].rearrange("(kc p) m -> p kc m", p=P))
            ps = mm_ps.tile([P, T], F32, tag="mm", name=f"ps2_{m}")
            for kc in range(MC_FC1):
                nc.tensor.matmul(ps[:], wt[:, kc, :], hbuf[:, kc, :],
                                 start=(kc == 0), stop=(kc == MC_FC1 - 1))
            po = tmp.tile([P, T], F32R, tag="po", name=f"po2_{m}")
            nc.scalar.activation(po[:], ps[:], AF.Identity, bias=b2_s[:, m:m + 1])
            nc.vector.tensor_add(ybuf[:, m, :], po[:], x1[:, m, :])
        nc.sync.dma_start(out_d.ap().rearrange("(kc p) t -> p kc t", p=P), ybuf[:])
        pH.release()
        pY.release()
        pW2.release()
        pX1.release()
        wpool.release()
        small.release()
        tmp.release()
        root.release()
        b3_ps.release()
        b2_ps.release()
        mm_ps.release()

    return nc


# ===================== host side =====================

_CACHE = {}


def _prep_shared(ln1_w, ln1_b, qkv_w, proj_w, proj_b, ln2_w, ln2_b,
                 fc1_w, fc1_b, fc2_w, fc2_b):
    f = np.float32
    scale = np.float32(HD ** -0.5)
    wq = (qkv_w * ln1_w[None, :]).astype(f)
    qkv_bias = (qkv_w @ ln1_b).astype(f)
    wq[:C] *= scale
    qkv_bias[:C] *= scale
    wqkv = np.ascontiguousarray(wq.T)                       # [C, 3C]
    qkvb = np.ascontiguousarray(qkv_bias[:2 * C].reshape(16, P).T)  # [128, 16]
    vbias = np.ascontiguousarray(qkv_bias[2 * C:].reshape(1, C))
    wproj = np.ascontiguousarray(proj_w.T.astype(f))
    projb = np.ascontiguousarray(proj_b.reshape(KC, P).T.astype(f))
    wfc1 = np.ascontiguousarray((fc1_w * ln2_w[None, :]).T.astype(f))
    fc1b = np.ascontiguousarray((fc1_w @ ln2_b + fc1_b).reshape(MC_FC1, P).T.astype(f))
    ones = np.ones((1, P), f)
    return dict(wqkv=wqkv, qkvb=qkvb, vbias=vbias, wproj=wproj, projb=projb,
                wfc1=wfc1, fc1b=fc1b, ones=ones)


def _get_runner():
    if "runner" in _CACHE:
        return _CACHE["runner"]
    import jax
    from jax.sharding import Mesh, PartitionSpec
    from jax.experimental.shard_map import shard_map
    from concourse import bass2jax, mybir as _mybir

    nc = build()
    nc.compile()
    bass2jax.install_neuronx_cc_hook()
    partition_name = nc.partition_id_tensor.name if nc.partition_id_tensor else None
    in_names, out_names, out_avals, zero_outs = [], [], [], []
    for alloc in nc.m.functions[0].allocations:
        if not isinstance(alloc, _mybir.MemoryLocationSet):
            continue
        name = alloc.memorylocations[0].name
        if alloc.kind == "ExternalInput":
            if name != partition_name:
                in_names.append(name)
        elif alloc.kind == "ExternalOutput":
            shape = tuple(alloc.tensor_shape)
            dtype = _mybir.dt.np(alloc.dtype)
            out_names.append(name)
            out_avals.append(jax.core.ShapedArray(shape, dtype))
            zero_outs.append(np.zeros(shape, dtype))
    n_params = len(in_names)
    n_outs = len(out_avals)
    all_names = list(in_names) + list(out_names)
    if partition_name is not None:
        all_names.append(partition_name)

    def _body(*args):
        operands = list(args)
        if partition_name is not None:
            operands.append(bass2jax.partition_id_tensor())
        outs = bass2jax._bass_exec_p.bind(
            *operands,
            out_avals=tuple(out_avals),
            in_names=tuple(all_names),
            out_names=tuple(out_names),
            lowering_input_output_aliases=(),
            sim_require_finite=True,
            sim_require_nnan=True,
            nc=nc,
        )
        return tuple(outs)

    n_cores = 8
    devices = jax.devices()[:n_cores]
    mesh = Mesh(np.asarray(devices), ("core",))
    in_specs = (PartitionSpec("core"),) * (n_params + n_outs)
    out_specs = (PartitionSpec("core"),) * n_outs
    sharded = jax.jit(
        shard_map(_body, mesh=mesh, in_specs=in_specs, out_specs=out_specs, check_rep=False),
        donate_argnums=tuple(range(n_params, n_params + n_outs)),
        keep_unused=True,
    )

    def run(in_maps):
        concat_in = [np.concatenate([np.asarray(in_maps[c][nm]) for c in range(n_cores)], axis=0)
                     for nm in in_names]
        concat_zeros = [np.zeros((n_cores * z.shape[0], *z.shape[1:]), z.dtype) for z in zero_outs]
        out_arrs = sharded(*concat_in, *concat_zeros)
        return [
            {nm: np.asarray(out_arrs[i]).reshape(n_cores, *out_avals[i].shape)[c]
             for i, nm in enumerate(out_names)}
            for c in range(n_cores)
        ]

    _CACHE["runner"] = run
    return run


def make_in_maps(x, indices, ln1_w, ln1_b, qkv_w, proj_w, proj_b,
                 ln2_w, ln2_b, fc1_w, fc1_b, fc2_w, fc2_b, exp_w, exp_b):
    x = np.asarray(x, np.float32)
    indices = np.asarray(indices)
    shared = _prep_shared(
        np.asarray(ln1_w, np.float32), np.asarray(ln1_b, np.float32),
        np.asarray(qkv_w, np.float32), np.asarray(proj_w, np.float32),
        np.asarray(proj_b, np.float32), np.asarray(ln2_w, np.float32),
        np.asarray(ln2_b, np.float32), np.asarray(fc1_w, np.float32),
        np.asarray(fc1_b, np.float32), np.asarray(fc2_w, np.float32),
        np.asarray(fc2_b, np.float32))
    fc2_w = np.asarray(fc2_w, np.float32)
    fc2_b = np.asarray(fc2_b, np.float32)
    exp_w = np.asarray(exp_w, np.float32)
    exp_b = np.asarray(exp_b, np.float32)

    in_maps = []
    for core in range(8):
        b, half = core // 2, core % 2
        # roll tokens so this core's half occupies columns [0, T); attention
        # is permutation-invariant over keys so K/V order doesn't matter
        xb = x[b]
        if half:
            xb = np.concatenate([xb[T:], xb[:T]], axis=0)
        xt = np.ascontiguousarray(xb.T)
        e = int(indices[b])
        w2 = np.ascontiguousarray(np.concatenate([fc2_w, exp_w[e]], axis=0).T)
        b2 = np.ascontiguousarray(
            np.concatenate([fc2_b, exp_b[e]]).reshape(KC, P).T)
        m = dict(shared)
        m["xt"] = xt
        m["w2"] = w2
        m["b2"] = b2
        in_maps.append(m)
    return in_maps


def assemble_output(outs):
    y = np.empty((B, N, C), np.float32)
    for core in range(8):
        b, half = core // 2, core % 2
        y[b, half * T:(half + 1) * T] = outs[core]["out"].T
    return y


def kernel(x, indices, ln1_w, ln1_b, qkv_w, proj_w, proj_b,
           ln2_w, ln2_b, fc1_w, fc1_b, fc2_w, fc2_b, exp_w, exp_b):
    in_maps = make_in_maps(x, indices, ln1_w, ln1_b, qkv_w, proj_w, proj_b,
                           ln2_w, ln2_b, fc1_w, fc1_b, fc2_w, fc2_b, exp_w, exp_b)
    run = _get_runner()
    outs = run(in_maps)
    return assemble_output(outs)


# revision 13
# speedup vs baseline: 1.1330x; 1.1330x over previous
"""Trainium2 Bass kernel for nn_Block_6433861009667 (transformer block w/ MoE MLP).

Sharding: 8 cores = (batch sample b = core//2) x (token half = core%2).
Each core computes its 512 tokens fully locally (K/V computed for the whole
sample on both cores of a pair -> no collectives at all).

Device layout: feature-major activations [C(partitions), T(free)], fp32r
matmuls (full PE rate, ~1.4e-4 matmul error), softmax denominators computed
on the PE via packed M=1 ones-matmuls, LayerNorm scale/shift folded into the
following matmul weights on the host.
"""
import sys

if "/opt/trn_rl_repo" not in sys.path:
    sys.path.insert(0, "/opt/trn_rl_repo")

import numpy as np

import concourse.bass as bass
import concourse.tile as tile
from concourse import bacc, mybir
from concourse.bass import ts

B, N, C = 4, 1024, 1024
H, HD = 16, 64
HID = 4 * C
PF, SHARED = 256, C - 256
EPS = 1e-5
T = 512            # tokens per core
P = 128
KC = C // P        # 8 feature chunks
MC_FC1 = HID // P  # 32
F32R = mybir.dt.float32r
F32 = mybir.dt.float32
AF = mybir.ActivationFunctionType


def build():
    nc = bacc.Bacc(trn_type="TRN2")

    # ---- DRAM I/O (per core) ----
    xt_d = nc.dram_tensor("xt", [C, N], F32R, kind="ExternalInput")          # x[b].T (tokens rolled so own half = cols 0:T)
    wqkv_d = nc.dram_tensor("wqkv", [C, 3 * C], F32R, kind="ExternalInput")  # folded (qkv_w*ln1w).T, Q cols pre-scaled
    qkvb_d = nc.dram_tensor("qkvb", [P, 16], F32R, kind="ExternalInput")     # Q/K bias per m-chunk column
    vbias_d = nc.dram_tensor("vbias", [1, C], F32R, kind="ExternalInput")    # V bias row
    wproj_d = nc.dram_tensor("wproj", [C, C], F32R, kind="ExternalInput")    # proj_w.T
    projb_d = nc.dram_tensor("projb", [P, KC], F32R, kind="ExternalInput")
    wfc1_d = nc.dram_tensor("wfc1", [C, HID], F32R, kind="ExternalInput")    # (fc1_w*ln2w).T
    fc1b_d = nc.dram_tensor("fc1b", [P, MC_FC1], F32R, kind="ExternalInput")
    w2_d = nc.dram_tensor("w2", [HID, C], F32R, kind="ExternalInput")        # concat([fc2_w, exp_w[b]]).T
    b2_d = nc.dram_tensor("b2", [P, KC], F32R, kind="ExternalInput")
    ones_d = nc.dram_tensor("ones", [1, P], F32R, kind="ExternalInput")
    out_d = nc.dram_tensor("out", [C, T], F32, kind="ExternalOutput")        # y[b, half].T

    with tile.TileContext(nc, pool_alloc_mode="queue") as tc:
        # ---- whole-kernel pools ----
        root = tc.alloc_tile_pool(name="root", bufs=1)
        tmp = tc.alloc_tile_pool(name="tmp", bufs=2)
        small = tc.alloc_tile_pool(name="small", bufs=1)
        wpool = tc.alloc_tile_pool(name="w", bufs=4)
        # PSUM budget: 3 + 2 + 2 = 7 banks of 8
        mm_ps = tc.alloc_tile_pool(name="mmps", bufs=3, space="PSUM")   # dense mm + attn S
        b2_ps = tc.alloc_tile_pool(name="b2ps", bufs=2, space="PSUM")   # attn O + LN stats
        b3_ps = tc.alloc_tile_pool(name="b3ps", bufs=2, space="PSUM")   # attn D + replicate

        # ---- constants / biases ----
        ones_sq = root.tile([P, P], F32R, tag="ones_sq")       # all-ones
        nc.sync.dma_start(ones_sq[:], ones_d.ap()[0, :].partition_broadcast(P))
        ones_col = root.tile([P, 1], F32R, tag="ones_col")
        nc.sync.dma_start(ones_col[:], ones_d.ap()[0, 0:1].partition_broadcast(P))
        qkvb_s = root.tile([P, 16], F32R, tag="qkvb")
        nc.sync.dma_start(qkvb_s[:], qkvb_d.ap())
        vbias_s = root.tile([1, C], F32R, tag="vbias")
        nc.sync.dma_start(vbias_s[:], vbias_d.ap())
        projb_s = root.tile([P, KC], F32R, tag="projb")
        nc.sync.dma_start(projb_s[:], projb_d.ap())
        fc1b_s = root.tile([P, MC_FC1], F32R, tag="fc1b")
        nc.sync.dma_start(fc1b_s[:], fc1b_d.ap())
        b2_s = root.tile([P, KC], F32R, tag="b2")
        nc.sync.dma_start(b2_s[:], b2_d.ap())
        eps_t = root.tile([1, 1], F32, tag="eps")
        nc.vector.memset(eps_t[:], EPS)

        # =============== helper: feature-major LayerNorm ===============
        def layernorm_fm(src, dst, tokens, aB, cB):
            """src/dst: [P, KC, tokens] f32r tiles. dst = (src - mu)*rstd per token."""
            halves = tokens // T
            negmu = small.tile([1, N], F32R, tag="lnm", name=f"negmu{tokens}")
            msq = small.tile([1, N], F32R, tag="lns", name=f"msq{tokens}")
            crow = small.tile([1, N], F32R, tag="lnc", name=f"crow{tokens}")
            for h in range(halves):
                sum_ps = b2_ps.tile([1, T], F32, tag="bank2", name=f"sum_ps{tokens}_{h}")
                sq_ps = b2_ps.tile([1, T], F32, tag="bank2", name=f"sq_ps{tokens}_{h}")
                for kc in range(KC):
                    xsq = tmp.tile([P, T], F32R, tag="xsq", name=f"xsq{tokens}_{h}_{kc}")
                    nc.vector.tensor_mul(xsq[:], src[:, kc, ts(h, T)], src[:, kc, ts(h, T)])
                    nc.tensor.matmul(sum_ps[:], ones_col[:], src[:, kc, ts(h, T)],
                                     start=(kc == 0), stop=(kc == KC - 1))
                    nc.tensor.matmul(sq_ps[:], ones_col[:], xsq[:],
                                     start=(kc == 0), stop=(kc == KC - 1))
                nc.scalar.mul(negmu[:, ts(h, T)], sum_ps[:], -1.0 / C)
                nc.scalar.mul(msq[:, ts(h, T)], sq_ps[:], 1.0 / C)
            tk = slice(0, tokens)
            nc.vector.tensor_mul(crow[:, tk], negmu[:, tk], negmu[:, tk])
            nc.vector.tensor_sub(msq[:, tk], msq[:, tk], crow[:, tk])
            # rstd = exp(-0.5*ln(var+eps))  (stays inside the exp/ln ACT table set)
            nc.scalar.activation(msq[:, tk], msq[:, tk], AF.Ln, bias=eps_t[0:1, :])
            nc.scalar.activation(msq[:, tk], msq[:, tk], AF.Exp, scale=-0.5)
            nc.vector.tensor_mul(crow[:, tk], negmu[:, tk], msq[:, tk])
            for h in range(halves):
                ra = b3_ps.tile([P, T], F32, tag="bank3", name=f"ra{tokens}_{h}")
                nc.tensor.matmul(ra[:], ones_sq[0:1, :], msq[:, ts(h, T)], start=True, stop=True)
                nc.scalar.copy(aB[:, ts(h, T)], ra[:])
                rc = b3_ps.tile([P, T], F32, tag="bank3", name=f"rc{tokens}_{h}")
                nc.tensor.matmul(rc[:], ones_sq[0:1, :], crow[:, ts(h, T)], start=True, stop=True)
                nc.scalar.copy(cB[:, ts(h, T)], rc[:])
            for kc in range(KC):
                nc.vector.tensor_mul(dst[:, kc, :], src[:, kc, :], aB[:])
                nc.vector.tensor_add(dst[:, kc, :], dst[:, kc, :], cB[:])

        # =============== LN1 ===============
        pLN1 = tc.alloc_tile_pool(name="pLN1", bufs=1)    # ln1 (until V done)
        ln1 = pLN1.tile([P, KC, N], F32R, tag="ln1")
        pWV = tc.alloc_tile_pool(name="pWV", bufs=1)      # wv (until V done)
        wv_s = pWV.tile([P, KC, C], F32R, tag="wv")
        nc.sync.dma_start(wv_s[:], wqkv_d.ap()[:, 2 * C:3 * C].rearrange("(kc p) m -> p kc m", p=P))

        pXT = tc.alloc_tile_pool(name="pXT", bufs=1)      # xt + LN1 broadcast rows
        xt_s = pXT.tile([P, KC, N], F32R, tag="xt")
        nc.sync.dma_start(xt_s[:], xt_d.ap().rearrange("(kc p) t -> p kc t", p=P))
        aB1 = pXT.tile([P, N], F32R, tag="aB1")
        cB1 = pXT.tile([P, N], F32R, tag="cB1")
        layernorm_fm(xt_s, ln1, N, aB1, cB1)
        pXT.release()

        # =============== QKV ===============
        pQ = tc.alloc_tile_pool(name="pQ", bufs=1, side="right")
        pK = tc.alloc_tile_pool(name="pK", bufs=1, side="right")
        pV = tc.alloc_tile_pool(name="pV", bufs=1, side="right")
        q_s = pQ.tile([P, KC, T], F32R, tag="q")
        k_s = pK.tile([P, KC, N], F32R, tag="k")
        vp = pV.tile([P, KC, C], F32R, tag="vp")

        # Q: out[m, t] for this core's (pre-rolled) token half = cols 0:T
        for m in range(KC):
            wt = wpool.tile([P, KC, P], F32R, tag="w128", name=f"wq{m}")
            nc.sync.dma_start(wt[:], wqkv_d.ap()[:, ts(m, P)].rearrange("(kc p) m -> p kc m", p=P))
            ps = mm_ps.tile([P, T], F32, tag="mm", name=f"psq{m}")
            for kc in range(KC):
                nc.tensor.matmul(ps[:], wt[:, kc, :], ln1[:, kc, 0:T],
                                 start=(kc == 0), stop=(kc == KC - 1))
            nc.scalar.activation(q_s[:, m, :], ps[:], AF.Identity, bias=qkvb_s[:, m:m + 1])
        # K: out[m, all N tokens]
        for m in range(KC):
            wt = wpool.tile([P, KC, P], F32R, tag="w128", name=f"wk{m}")
            nc.sync.dma_start(wt[:], wqkv_d.ap()[:, C + m * P:C + (m + 1) * P].rearrange("(kc p) m -> p kc m", p=P))
            for h in range(2):
                ps = mm_ps.tile([P, T], F32, tag="mm", name=f"psk{m}_{h}")
                for kc in range(KC):
                    nc.tensor.matmul(ps[:], wt[:, kc, :], ln1[:, kc, ts(h, T)],
                                     start=(kc == 0), stop=(kc == KC - 1))
                nc.scalar.activation(k_s[:, m, ts(h, T)], ps[:], AF.Identity,
                                     bias=qkvb_s[:, 8 + m:9 + m])
        # V (token-major): out[j, d] ; lhsT = ln1 chunk (j slice), rhs = wv
        for jc in range(KC):
            for dh in range(2):
                ps = mm_ps.tile([P, T], F32, tag="mm", name=f"psv{jc}_{dh}")
                for kc in range(KC):
                    nc.tensor.matmul(ps[:], ln1[:, kc, ts(jc, P)], wv_s[:, kc, ts(dh, T)],
                                     start=(kc == 0), stop=False)
                nc.tensor.matmul(ps[:], ones_sq[0:1, :], vbias_s[:, ts(dh, T)],
                                 start=False, stop=True)
                nc.scalar.copy(vp[:, jc, ts(dh, T)], ps[:])
        pWV.release()
        pLN1.release()

        # =============== Attention (head pairs) ===============
        pX1 = tc.alloc_tile_pool(name="pX1", bufs=1)      # x1 (until end; below pATT on left stack)
        pATT = tc.alloc_tile_pool(name="pATT", bufs=1)    # attn (until proj done)
        attn = pATT.tile([P, KC, T], F32R, tag="attn")
        pPH = tc.alloc_tile_pool(name="pPH", bufs=16, side="right")      # per-(head,jc) P^T tiles

        def s_mms(c):
            phs = [[None] * KC, [None] * KC]
            for jc in range(KC):
                for g in range(2):
                    b0 = g * 64
                    ps = mm_ps.tile([P, T], F32, tag="mm", name=f"s{c}_{g}_{jc}")
                    nc.tensor.matmul(ps[:], k_s[b0:b0 + 64, c, ts(jc, P)],
                                     q_s[b0:b0 + 64, c, :], start=True, stop=True)
                    ph = pPH.tile([P, T], F32R, tag="ph", name=f"ph{c}_{g}_{jc}")
                    nc.scalar.activation(ph[:], ps[:], AF.Exp)
                    phs[g][jc] = ph
            return phs

        def pv_mms(c, phs):
            for g in range(2):
                h = 2 * c + g
                pso = b2_ps.tile([64, T], F32, tag="bank2", name=f"pso{c}_{g}")
                psd_lo = b3_ps.tile([1, T], F32, tag="bank3", name=f"psdl{c}_{g}")
                psd_hi = b3_ps.tile([1, T], F32, tag="bank3", name=f"psdh{c}_{g}")
                for jc in range(KC):
                    st = (jc == 0)
                    sp = (jc == KC - 1)
                    nc.tensor.matmul(pso[:], vp[:, jc, h * 64:(h + 1) * 64],
                                     phs[g][jc][:], start=st, stop=sp)
                    # denominator split into two row-groups -> runs concurrent
                    nc.tensor.matmul(psd_lo[:], ones_col[0:64, :], phs[g][jc][0:64, :],
                                     start=st, stop=sp)
                    nc.tensor.matmul(psd_hi[:], ones_col[64:128, :], phs[g][jc][64:128, :],
                                     start=st, stop=sp)
                rcp = tmp.tile([1, T], F32R, tag="rcp", name=f"rcp{c}_{g}")
                nc.scalar.copy(rcp[:], psd_lo[:])
                with nc.allow_low_precision(reason="f32r bits are full fp32 here"):
                    nc.vector.tensor_add(rcp[:], rcp[:], psd_hi[:])
                    nc.vector.reciprocal(rcp[:], rcp[:])
                psr = b3_ps.tile([64, T], F32, tag="bank3", name=f"psr{c}_{g}")
                nc.tensor.matmul(psr[:], ones_sq[0:1, 0:64], rcp[0:1, :],
                                 start=True, stop=True)
                if g == 0:
                    nc.scalar.copy(attn[0:64, c, :], pso[:])
                    nc.vector.tensor_mul(attn[0:64, c, :], attn[0:64, c, :], psr[:])
                else:
                    ot = tmp.tile([64, T], F32R, tag="ot", name=f"ot{c}")
                    nc.scalar.copy(ot[:], pso[:])
                    nc.vector.tensor_mul(ot[:], ot[:], psr[:])
                    nc.sync.dma_start(attn[64:128, c, :], ot[:])

        prev = s_mms(0)
        for c in range(KC):
            cur = prev
            if c + 1 < KC:
                prev = s_mms(c + 1)
            pv_mms(c, cur)
        pPH.release()
        pV.release()
        pK.release()
        pQ.release()

        # =============== proj + residual ===============
        x1 = pX1.tile([P, KC, T], F32R, tag="x1")
        for m in range(KC):
            wt = wpool.tile([P, KC, P], F32R, tag="w128", name=f"wp{m}")
            nc.sync.dma_start(wt[:], wproj_d.ap()[:, ts(m, P)].rearrange("(kc p) m -> p kc m", p=P))
            xh = tmp.tile([P, T], F32R, tag="xh", name=f"xh{m}")
            nc.sync.dma_start(xh[:], xt_d.ap()[ts(m, P), 0:T])
            ps = mm_ps.tile([P, T], F32, tag="mm", name=f"psp{m}")
            for kc in range(KC):
                nc.tensor.matmul(ps[:], wt[:, kc, :], attn[:, kc, :],
                                 start=(kc == 0), stop=(kc == KC - 1))
            po = tmp.tile([P, T], F32R, tag="po", name=f"po{m}")
            nc.scalar.activation(po[:], ps[:], AF.Identity, bias=projb_s[:, m:m + 1])
            nc.vector.tensor_add(x1[:, m, :], po[:], xh[:])
        pATT.release()

        # =============== LN2 ===============
        pLN2 = tc.alloc_tile_pool(name="pLN2", bufs=1, side="right")
        ln2t = pLN2.tile([P, KC, T], F32R, tag="ln2t")
        aB2 = pLN2.tile([P, T], F32R, tag="aB2")
        cB2 = pLN2.tile([P, T], F32R, tag="cB2")
        layernorm_fm(x1, ln2t, T, aB2, cB2)

        # =============== FC1 + gelu ===============
        pH = tc.alloc_tile_pool(name="pH", bufs=1)
        hbuf = pH.tile([P, MC_FC1, T], F32R, tag="h")
        for m in range(MC_FC1):
            wt = wpool.tile([P, KC, P], F32R, tag="w128", name=f"wf{m}")
            nc.sync.dma_start(wt[:], wfc1_d.ap()[:, ts(m, P)].rearrange("(kc p) m -> p kc m", p=P))
            ps = mm_ps.tile([P, T], F32, tag="mm", name=f"psf{m}")
            for kc in range(KC):
                nc.tensor.matmul(ps[:], wt[:, kc, :], ln2t[:, kc, :],
                                 start=(kc == 0), stop=(kc == KC - 1))
            nc.scalar.activation(hbuf[:, m, :], ps[:], AF.Gelu, bias=fc1b_s[:, m:m + 1])
        pLN2.release()

        # =============== FC2 (shared+expert fused) + residual + out ===============
        pW2 = tc.alloc_tile_pool(name="pW2", bufs=3, side="right")
        pY = tc.alloc_tile_pool(name="pY", bufs=1, side="right")
        ybuf = pY.tile([P, KC, T], F32, tag="y")
        for m in range(KC):
            wt = pW2.tile([P, MC_FC1, P], F32R, tag="w2t", name=f"w2_{m}")
            nc.sync.dma_start(wt[:], w2_d.ap()[:, ts(m, P)].rearrange("(kc p) m -> p kc m", p=P))
            ps = mm_ps.tile([P, T], F32, tag="mm", name=f"ps2_{m}")
            for kc in range(MC_FC1):
                nc.tensor.matmul(ps[:], wt[:, kc, :], hbuf[:, kc, :],
                                 start=(kc == 0), stop=(kc == MC_FC1 - 1))
            po = tmp.tile([P, T], F32R, tag="po", name=f"po2_{m}")
            nc.scalar.activation(po[:], ps[:], AF.Identity, bias=b2_s[:, m:m + 1])
            nc.vector.tensor_add(ybuf[:, m, :], po[:], x1[:, m, :])
            nc.sync.dma_start(out_d.ap()[ts(m, P), :], ybuf[:, m, :])
        pH.release()
        pY.release()
        pW2.release()
        pX1.release()
        wpool.release()
        small.release()
        tmp.release()
        root.release()
        b3_ps.release()
        b2_ps.release()
        mm_ps.release()

    return nc


# ===================== host side =====================

_CACHE = {}


def _prep_shared(ln1_w, ln1_b, qkv_w, proj_w, proj_b, ln2_w, ln2_b,
                 fc1_w, fc1_b, fc2_w, fc2_b):
    f = np.float32
    scale = np.float32(HD ** -0.5)
    wq = (qkv_w * ln1_w[None, :]).astype(f)
    qkv_bias = (qkv_w @ ln1_b).astype(f)
    wq[:C] *= scale
    qkv_bias[:C] *= scale
    wqkv = np.ascontiguousarray(wq.T)                       # [C, 3C]
    qkvb = np.ascontiguousarray(qkv_bias[:2 * C].reshape(16, P).T)  # [128, 16]
    vbias = np.ascontiguousarray(qkv_bias[2 * C:].reshape(1, C))
    wproj = np.ascontiguousarray(proj_w.T.astype(f))
    projb = np.ascontiguousarray(proj_b.reshape(KC, P).T.astype(f))
    wfc1 = np.ascontiguousarray((fc1_w * ln2_w[None, :]).T.astype(f))
    fc1b = np.ascontiguousarray((fc1_w @ ln2_b + fc1_b).reshape(MC_FC1, P).T.astype(f))
    ones = np.ones((1, P), f)
    return dict(wqkv=wqkv, qkvb=qkvb, vbias=vbias, wproj=wproj, projb=projb,
                wfc1=wfc1, fc1b=fc1b, ones=ones)


def _get_runner():
    if "runner" in _CACHE:
        return _CACHE["runner"]
    import jax
    from jax.sharding import Mesh, PartitionSpec
    from jax.experimental.shard_map import shard_map
    from concourse import bass2jax, mybir as _mybir

    nc = build()
    nc.compile()
    bass2jax.install_neuronx_cc_hook()
    partition_name = nc.partition_id_tensor.name if nc.partition_id_tensor else None
    in_names, out_names, out_avals, zero_outs = [], [], [], []
    for alloc in nc.m.functions[0].allocations:
        if not isinstance(alloc, _mybir.MemoryLocationSet):
            continue
        name = alloc.memorylocations[0].name
        if alloc.kind == "ExternalInput":
            if name != partition_name:
                in_names.append(name)
        elif alloc.kind == "ExternalOutput":
            shape = tuple(alloc.tensor_shape)
            dtype = _mybir.dt.np(alloc.dtype)
            out_names.append(name)
            out_avals.append(jax.core.ShapedArray(shape, dtype))
            zero_outs.append(np.zeros(shape, dtype))
    n_params = len(in_names)
    n_outs = len(out_avals)
    all_names = list(in_names) + list(out_names)
    if partition_name is not None:
        all_names.append(partition_name)

    def _body(*args):
        operands = list(args)
        if partition_name is not None:
            operands.append(bass2jax.partition_id_tensor())
        outs = bass2jax._bass_exec_p.bind(
            *operands,
            out_avals=tuple(out_avals),
            in_names=tuple(all_names),
            out_names=tuple(out_names),
            lowering_input_output_aliases=(),
            sim_require_finite=True,
            sim_require_nnan=True,
            nc=nc,
        )
        return tuple(outs)

    n_cores = 8
    devices = jax.devices()[:n_cores]
    mesh = Mesh(np.asarray(devices), ("core",))
    in_specs = (PartitionSpec("core"),) * (n_params + n_outs)
    out_specs = (PartitionSpec("core"),) * n_outs
    sharded = jax.jit(
        shard_map(_body, mesh=mesh, in_specs=in_specs, out_specs=out_specs, check_rep=False),
        donate_argnums=tuple(range(n_params, n_params + n_outs)),
        keep_unused=True,
    )

    def run(in_maps):
        concat_in = [np.concatenate([np.asarray(in_maps[c][nm]) for c in range(n_cores)], axis=0)
                     for nm in in_names]
        concat_zeros = [np.zeros((n_cores * z.shape[0], *z.shape[1:]), z.dtype) for z in zero_outs]
        out_arrs = sharded(*concat_in, *concat_zeros)
        return [
            {nm: np.asarray(out_arrs[i]).reshape(n_cores, *out_avals[i].shape)[c]
             for i, nm in enumerate(out_names)}
            for c in range(n_cores)
        ]

    _CACHE["runner"] = run
    return run


def make_in_maps(x, indices, ln1_w, ln1_b, qkv_w, proj_w, proj_b,
                 ln2_w, ln2_b, fc1_w, fc1_b, fc2_w, fc2_b, exp_w, exp_b):
    x = np.asarray(x, np.float32)
    indices = np.asarray(indices)
    shared = _prep_shared(
        np.asarray(ln1_w, np.float32), np.asarray(ln1_b, np.float32),
        np.asarray(qkv_w, np.float32), np.asarray(proj_w, np.float32),
        np.asarray(proj_b, np.float32), np.asarray(ln2_w, np.float32),
        np.asarray(ln2_b, np.float32), np.asarray(fc1_w, np.float32),
        np.asarray(fc1_b, np.float32), np.asarray(fc2_w, np.float32),
        np.asarray(fc2_b, np.float32))
    fc2_w = np.asarray(fc2_w, np.float32)
    fc2_b = np.asarray(fc2_b, np.float32)
    exp_w = np.asarray(exp_w, np.float32)
    exp_b = np.asarray(exp_b, np.float32)

    in_maps = []
    for core in range(8):
        b, half = core // 2, core % 2
        # roll tokens so this core's half occupies columns [0, T); attention
        # is permutation-invariant over keys so K/V order doesn't matter
        xb = x[b]
        if half:
            xb = np.concatenate([xb[T:], xb[:T]], axis=0)
        xt = np.ascontiguousarray(xb.T)
        e = int(indices[b])
        w2 = np.ascontiguousarray(np.concatenate([fc2_w, exp_w[e]], axis=0).T)
        b2 = np.ascontiguousarray(
            np.concatenate([fc2_b, exp_b[e]]).reshape(KC, P).T)
        m = dict(shared)
        m["xt"] = xt
        m["w2"] = w2
        m["b2"] = b2
        in_maps.append(m)
    return in_maps


def assemble_output(outs):
    y = np.empty((B, N, C), np.float32)
    for core in range(8):
        b, half = core // 2, core % 2
        y[b, half * T:(half + 1) * T] = outs[core]["out"].T
    return y


def kernel(x, indices, ln1_w, ln1_b, qkv_w, proj_w, proj_b,
           ln2_w, ln2_b, fc1_w, fc1_b, fc2_w, fc2_b, exp_w, exp_b):
    in_maps = make_in_maps(x, indices, ln1_w, ln1_b, qkv_w, proj_w, proj_b,
                           ln2_w, ln2_b, fc1_w, fc1_b, fc2_w, fc2_b, exp_w, exp_b)
    run = _get_runner()
    outs = run(in_maps)
    return assemble_output(outs)


# revision 15
# speedup vs baseline: 1.1749x; 1.0370x over previous
"""Trainium2 Bass kernel for nn_Block_6433861009667 (transformer block w/ MoE MLP).

Sharding: 8 cores = (batch sample b = core//2) x (token half = core%2).
Each core computes its 512 tokens fully locally (K/V computed for the whole
sample on both cores of a pair -> no collectives at all).

Device layout: feature-major activations [C(partitions), T(free)], fp32r
matmuls (full PE rate, ~1.4e-4 matmul error), softmax denominators computed
on the PE via packed M=1 ones-matmuls, LayerNorm scale/shift folded into the
following matmul weights on the host.
"""
import sys

if "/opt/trn_rl_repo" not in sys.path:
    sys.path.insert(0, "/opt/trn_rl_repo")

import numpy as np

import concourse.bass as bass
import concourse.tile as tile
from concourse import bacc, mybir
from concourse.bass import ts

B, N, C = 4, 1024, 1024
H, HD = 16, 64
HID = 4 * C
PF, SHARED = 256, C - 256
EPS = 1e-5
T = 512            # tokens per core
P = 128
KC = C // P        # 8 feature chunks
MC_FC1 = HID // P  # 32
F32R = mybir.dt.float32r
F32 = mybir.dt.float32
AF = mybir.ActivationFunctionType


def build():
    nc = bacc.Bacc(trn_type="TRN2")

    # ---- DRAM I/O (per core) ----
    xt_d = nc.dram_tensor("xt", [C, N], F32R, kind="ExternalInput")          # x[b].T (tokens rolled so own half = cols 0:T)
    wqkv_d = nc.dram_tensor("wqkv", [C, 3 * C], F32R, kind="ExternalInput")  # folded (qkv_w*ln1w).T, Q cols pre-scaled
    qkvb_d = nc.dram_tensor("qkvb", [P, 16], F32R, kind="ExternalInput")     # Q/K bias per m-chunk column
    vbias_d = nc.dram_tensor("vbias", [1, C], F32R, kind="ExternalInput")    # V bias row
    wproj_d = nc.dram_tensor("wproj", [C, C], F32R, kind="ExternalInput")    # proj_w.T
    projb_d = nc.dram_tensor("projb", [P, KC], F32R, kind="ExternalInput")
    wfc1_d = nc.dram_tensor("wfc1", [C, HID], F32R, kind="ExternalInput")    # (fc1_w*ln2w).T
    fc1b_d = nc.dram_tensor("fc1b", [P, MC_FC1], F32R, kind="ExternalInput")
    w2_d = nc.dram_tensor("w2", [HID, C], F32R, kind="ExternalInput")        # concat([fc2_w, exp_w[b]]).T
    b2_d = nc.dram_tensor("b2", [P, KC], F32R, kind="ExternalInput")
    ones_d = nc.dram_tensor("ones", [1, P], F32R, kind="ExternalInput")
    out_d = nc.dram_tensor("out", [C, T], F32, kind="ExternalOutput")        # y[b, half].T

    with tile.TileContext(nc, pool_alloc_mode="queue") as tc:
        # ---- whole-kernel pools ----
        root = tc.alloc_tile_pool(name="root", bufs=1)
        tmp = tc.alloc_tile_pool(name="tmp", bufs=2)
        small = tc.alloc_tile_pool(name="small", bufs=1)
        wpool = tc.alloc_tile_pool(name="w", bufs=4)
        # PSUM budget: 3 + 2 + 2 = 7 banks of 8
        mm_ps = tc.alloc_tile_pool(name="mmps", bufs=3, space="PSUM")   # dense mm + attn S
        b2_ps = tc.alloc_tile_pool(name="b2ps", bufs=2, space="PSUM")   # attn O + LN stats
        b3_ps = tc.alloc_tile_pool(name="b3ps", bufs=2, space="PSUM")   # attn D + replicate

        # ---- constants / biases ----
        ones_sq = root.tile([P, P], F32R, tag="ones_sq")       # all-ones
        nc.sync.dma_start(ones_sq[:], ones_d.ap()[0, :].partition_broadcast(P))
        ones_col = root.tile([P, 1], F32R, tag="ones_col")
        nc.sync.dma_start(ones_col[:], ones_d.ap()[0, 0:1].partition_broadcast(P))
        qkvb_s = root.tile([P, 16], F32R, tag="qkvb")
        nc.sync.dma_start(qkvb_s[:], qkvb_d.ap())
        vbias_s = root.tile([1, C], F32R, tag="vbias")
        nc.sync.dma_start(vbias_s[:], vbias_d.ap())
        projb_s = root.tile([P, KC], F32R, tag="projb")
        nc.sync.dma_start(projb_s[:], projb_d.ap())
        fc1b_s = root.tile([P, MC_FC1], F32R, tag="fc1b")
        nc.sync.dma_start(fc1b_s[:], fc1b_d.ap())
        b2_s = root.tile([P, KC], F32R, tag="b2")
        nc.sync.dma_start(b2_s[:], b2_d.ap())
        eps_t = root.tile([1, 1], F32, tag="eps")
        nc.vector.memset(eps_t[:], EPS)

        # =============== helper: feature-major LayerNorm ===============
        def layernorm_fm(src, dst, tokens, aB, cB):
            """src/dst: [P, KC, tokens] f32r tiles. dst = (src - mu)*rstd per token."""
            halves = tokens // T
            negmu = small.tile([1, N], F32R, tag="lnm", name=f"negmu{tokens}")
            msq = small.tile([1, N], F32R, tag="lns", name=f"msq{tokens}")
            crow = small.tile([1, N], F32R, tag="lnc", name=f"crow{tokens}")
            for h in range(halves):
                sum_ps = b2_ps.tile([1, T], F32, tag="bank2", name=f"sum_ps{tokens}_{h}")
                sq_ps = b2_ps.tile([1, T], F32, tag="bank2", name=f"sq_ps{tokens}_{h}")
                for kc in range(KC):
                    xsq = tmp.tile([P, T], F32R, tag="xsq", name=f"xsq{tokens}_{h}_{kc}")
                    nc.vector.tensor_mul(xsq[:], src[:, kc, ts(h, T)], src[:, kc, ts(h, T)])
                    nc.tensor.matmul(sum_ps[:], ones_col[:], src[:, kc, ts(h, T)],
                                     start=(kc == 0), stop=(kc == KC - 1))
                    nc.tensor.matmul(sq_ps[:], ones_col[:], xsq[:],
                                     start=(kc == 0), stop=(kc == KC - 1))
                nc.scalar.mul(negmu[:, ts(h, T)], sum_ps[:], -1.0 / C)
                nc.scalar.mul(msq[:, ts(h, T)], sq_ps[:], 1.0 / C)
            tk = slice(0, tokens)
            nc.vector.tensor_mul(crow[:, tk], negmu[:, tk], negmu[:, tk])
            nc.vector.tensor_sub(msq[:, tk], msq[:, tk], crow[:, tk])
            # rstd = exp(-0.5*ln(var+eps))  (stays inside the exp/ln ACT table set)
            nc.scalar.activation(msq[:, tk], msq[:, tk], AF.Ln, bias=eps_t[0:1, :])
            nc.scalar.activation(msq[:, tk], msq[:, tk], AF.Exp, scale=-0.5)
            nc.vector.tensor_mul(crow[:, tk], negmu[:, tk], msq[:, tk])
            for h in range(halves):
                ra = b3_ps.tile([P, T], F32, tag="bank3", name=f"ra{tokens}_{h}")
                nc.tensor.matmul(ra[:], ones_sq[0:1, :], msq[:, ts(h, T)], start=True, stop=True)
                nc.scalar.copy(aB[:, ts(h, T)], ra[:])
                rc = b3_ps.tile([P, T], F32, tag="bank3", name=f"rc{tokens}_{h}")
                nc.tensor.matmul(rc[:], ones_sq[0:1, :], crow[:, ts(h, T)], start=True, stop=True)
                nc.scalar.copy(cB[:, ts(h, T)], rc[:])
            for kc in range(KC):
                nc.vector.tensor_mul(dst[:, kc, :], src[:, kc, :], aB[:])
                nc.vector.tensor_add(dst[:, kc, :], dst[:, kc, :], cB[:])

        # =============== LN1 ===============
        pLN1 = tc.alloc_tile_pool(name="pLN1", bufs=1)    # ln1 (until V done)
        ln1 = pLN1.tile([P, KC, N], F32R, tag="ln1")
        pWV = tc.alloc_tile_pool(name="pWV", bufs=1)      # wv (until V done)
        wv_s = pWV.tile([P, KC, C], F32R, tag="wv")
        nc.sync.dma_start(wv_s[:], wqkv_d.ap()[:, 2 * C:3 * C].rearrange("(kc p) m -> p kc m", p=P))

        pXT = tc.alloc_tile_pool(name="pXT", bufs=1)      # xt + LN1 broadcast rows
        xt_s = pXT.tile([P, KC, N], F32R, tag="xt")
        nc.sync.dma_start(xt_s[:], xt_d.ap().rearrange("(kc p) t -> p kc t", p=P))
        aB1 = pXT.tile([P, N], F32R, tag="aB1")
        cB1 = pXT.tile([P, N], F32R, tag="cB1")
        layernorm_fm(xt_s, ln1, N, aB1, cB1)
        pXT.release()

        # =============== QKV ===============
        pQ = tc.alloc_tile_pool(name="pQ", bufs=1, side="right")
        pK = tc.alloc_tile_pool(name="pK", bufs=1, side="right")
        pV = tc.alloc_tile_pool(name="pV", bufs=1, side="right")
        q_s = pQ.tile([P, KC, T], F32R, tag="q")
        k_s = pK.tile([P, KC, N], F32R, tag="k")
        vp = pV.tile([P, KC, H, HD + 1], F32R, tag="vp")

        # Q: out[m, t] for this core's (pre-rolled) token half = cols 0:T
        for m in range(KC):
            wt = wpool.tile([P, KC, P], F32R, tag="w128", name=f"wq{m}")
            nc.sync.dma_start(wt[:], wqkv_d.ap()[:, ts(m, P)].rearrange("(kc p) m -> p kc m", p=P))
            ps = mm_ps.tile([P, T], F32, tag="mm", name=f"psq{m}")
            for kc in range(KC):
                nc.tensor.matmul(ps[:], wt[:, kc, :], ln1[:, kc, 0:T],
                                 start=(kc == 0), stop=(kc == KC - 1))
            nc.scalar.activation(q_s[:, m, :], ps[:], AF.Identity, bias=qkvb_s[:, m:m + 1])
        # K: out[m, all N tokens]
        for m in range(KC):
            wt = wpool.tile([P, KC, P], F32R, tag="w128", name=f"wk{m}")
            nc.sync.dma_start(wt[:], wqkv_d.ap()[:, C + m * P:C + (m + 1) * P].rearrange("(kc p) m -> p kc m", p=P))
            for h in range(2):
                ps = mm_ps.tile([P, T], F32, tag="mm", name=f"psk{m}_{h}")
                for kc in range(KC):
                    nc.tensor.matmul(ps[:], wt[:, kc, :], ln1[:, kc, ts(h, T)],
                                     start=(kc == 0), stop=(kc == KC - 1))
                nc.scalar.activation(k_s[:, m, ts(h, T)], ps[:], AF.Identity,
                                     bias=qkvb_s[:, 8 + m:9 + m])
        # V (token-major): out[j, d] ; lhsT = ln1 chunk (j slice), rhs = wv
        for jc in range(KC):
            for dh in range(2):
                ps = mm_ps.tile([P, T], F32, tag="mm", name=f"psv{jc}_{dh}")
                for kc in range(KC):
                    nc.tensor.matmul(ps[:], ln1[:, kc, ts(jc, P)], wv_s[:, kc, ts(dh, T)],
                                     start=(kc == 0), stop=False)
                nc.tensor.matmul(ps[:], ones_sq[0:1, :], vbias_s[:, ts(dh, T)],
                                 start=False, stop=True)
                nc.scalar.copy(vp[:, jc, dh * 8:(dh + 1) * 8, 0:HD],
                               ps[:].rearrange("p (g d) -> p g d", g=8))
        for jc in range(KC):
            nc.scalar.copy(vp[:, jc, :, HD:HD + 1], ones_sq[:, 0:H, None])
        pWV.release()
        pLN1.release()

        # =============== Attention (head pairs) ===============
        pX1 = tc.alloc_tile_pool(name="pX1", bufs=1)      # x1 (until end; below pATT on left stack)
        pATT = tc.alloc_tile_pool(name="pATT", bufs=1)    # attn (until proj done)
        attn = pATT.tile([P, KC, T], F32R, tag="attn")
        pPH = tc.alloc_tile_pool(name="pPH", bufs=16, side="right")      # per-(head,jc) P^T tiles

        def s_mms(c):
            phs = [[None] * KC, [None] * KC]
            for jc in range(KC):
                for g in range(2):
                    b0 = g * 64
                    ps = mm_ps.tile([P, T], F32, tag="mm", name=f"s{c}_{g}_{jc}")
                    nc.tensor.matmul(ps[:], k_s[b0:b0 + 64, c, ts(jc, P)],
                                     q_s[b0:b0 + 64, c, :], start=True, stop=True)
                    ph = pPH.tile([P, T], F32R, tag="ph", name=f"ph{c}_{g}_{jc}")
                    nc.scalar.activation(ph[:], ps[:], AF.Exp)
                    phs[g][jc] = ph
            return phs

        def pv_mms(c, phs):
            for g in range(2):
                h = 2 * c + g
                pso = b2_ps.tile([HD + 1, T], F32, tag="bank2", name=f"pso{c}_{g}")
                for jc in range(KC):
                    nc.tensor.matmul(pso[:], vp[:, jc, h, :], phs[g][jc][:],
                                     start=(jc == 0), stop=(jc == KC - 1))
                rcp = tmp.tile([HD + 1, T], F32R, tag="rcp", name=f"rcp{c}_{g}")
                nc.scalar.copy(rcp[HD:HD + 1, :], pso[HD:HD + 1, :])
                with nc.allow_low_precision(reason="f32r bits are full fp32 here"):
                    nc.vector.reciprocal(rcp[HD:HD + 1, :], rcp[HD:HD + 1, :])
                psr = b3_ps.tile([HD, T], F32, tag="bank3", name=f"psr{c}_{g}")
                nc.tensor.matmul(psr[:], ones_sq[HD:HD + 1, 0:HD], rcp[HD:HD + 1, :],
                                 start=True, stop=True)
                if g == 0:
                    nc.scalar.copy(attn[0:HD, c, :], pso[0:HD, :])
                    nc.vector.tensor_mul(attn[0:HD, c, :], attn[0:HD, c, :], psr[:])
                else:
                    ot = tmp.tile([HD, T], F32R, tag="ot", name=f"ot{c}")
                    nc.scalar.copy(ot[:], pso[0:HD, :])
                    nc.vector.tensor_mul(ot[:], ot[:], psr[:])
                    nc.sync.dma_start(attn[HD:P, c, :], ot[:])

        prev = s_mms(0)
        for c in range(KC):
            cur = prev
            if c + 1 < KC:
                prev = s_mms(c + 1)
            pv_mms(c, cur)
        pPH.release()
        pV.release()
        pK.release()
        pQ.release()

        # =============== proj + residual ===============
        x1 = pX1.tile([P, KC, T], F32R, tag="x1")
        for m in range(KC):
            wt = wpool.tile([P, KC, P], F32R, tag="w128", name=f"wp{m}")
            nc.sync.dma_start(wt[:], wproj_d.ap()[:, ts(m, P)].rearrange("(kc p) m -> p kc m", p=P))
            xh = tmp.tile([P, T], F32R, tag="xh", name=f"xh{m}")
            nc.sync.dma_start(xh[:], xt_d.ap()[ts(m, P), 0:T])
            ps = mm_ps.tile([P, T], F32, tag="mm", name=f"psp{m}")
            for kc in range(KC):
                nc.tensor.matmul(ps[:], wt[:, kc, :], attn[:, kc, :],
                                 start=(kc == 0), stop=(kc == KC - 1))
            po = tmp.tile([P, T], F32R, tag="po", name=f"po{m}")
            nc.scalar.activation(po[:], ps[:], AF.Identity, bias=projb_s[:, m:m + 1])
            nc.vector.tensor_add(x1[:, m, :], po[:], xh[:])
        pATT.release()

        # =============== LN2 ===============
        pLN2 = tc.alloc_tile_pool(name="pLN2", bufs=1, side="right")
        ln2t = pLN2.tile([P, KC, T], F32R, tag="ln2t")
        aB2 = pLN2.tile([P, T], F32R, tag="aB2")
        cB2 = pLN2.tile([P, T], F32R, tag="cB2")
        layernorm_fm(x1, ln2t, T, aB2, cB2)

        # =============== FC1 + gelu ===============
        pH = tc.alloc_tile_pool(name="pH", bufs=1)
        hbuf = pH.tile([P, MC_FC1, T], F32R, tag="h")
        for m in range(MC_FC1):
            wt = wpool.tile([P, KC, P], F32R, tag="w128", name=f"wf{m}")
            nc.sync.dma_start(wt[:], wfc1_d.ap()[:, ts(m, P)].rearrange("(kc p) m -> p kc m", p=P))
            ps = mm_ps.tile([P, T], F32, tag="mm", name=f"psf{m}")
            for kc in range(KC):
                nc.tensor.matmul(ps[:], wt[:, kc, :], ln2t[:, kc, :],
                                 start=(kc == 0), stop=(kc == KC - 1))
            nc.scalar.activation(hbuf[:, m, :], ps[:], AF.Gelu, bias=fc1b_s[:, m:m + 1])
        pLN2.release()

        # =============== FC2 (shared+expert fused) + residual + out ===============
        pW2 = tc.alloc_tile_pool(name="pW2", bufs=3, side="right")
        pY = tc.alloc_tile_pool(name="pY", bufs=1, side="right")
        ybuf = pY.tile([P, KC, T], F32, tag="y")
        for m in range(KC):
            wt = pW2.tile([P, MC_FC1, P], F32R, tag="w2t", name=f"w2_{m}")
            nc.sync.dma_start(wt[:], w2_d.ap()[:, ts(m, P)].rearrange("(kc p) m -> p kc m", p=P))
            ps = mm_ps.tile([P, T], F32, tag="mm", name=f"ps2_{m}")
            for kc in range(MC_FC1):
                nc.tensor.matmul(ps[:], wt[:, kc, :], hbuf[:, kc, :],
                                 start=(kc == 0), stop=(kc == MC_FC1 - 1))
            po = tmp.tile([P, T], F32R, tag="po", name=f"po2_{m}")
            nc.scalar.activation(po[:], ps[:], AF.Identity, bias=b2_s[:, m:m + 1])
            nc.vector.tensor_add(ybuf[:, m, :], po[:], x1[:, m, :])
            nc.sync.dma_start(out_d.ap()[ts(m, P), :], ybuf[:, m, :])
        pH.release()
        pY.release()
        pW2.release()
        pX1.release()
        wpool.release()
        small.release()
        tmp.release()
        root.release()
        b3_ps.release()
        b2_ps.release()
        mm_ps.release()

    return nc


# ===================== host side =====================

_CACHE = {}


def _prep_shared(ln1_w, ln1_b, qkv_w, proj_w, proj_b, ln2_w, ln2_b,
                 fc1_w, fc1_b, fc2_w, fc2_b):
    f = np.float32
    scale = np.float32(HD ** -0.5)
    wq = (qkv_w * ln1_w[None, :]).astype(f)
    qkv_bias = (qkv_w @ ln1_b).astype(f)
    wq[:C] *= scale
    qkv_bias[:C] *= scale
    wqkv = np.ascontiguousarray(wq.T)                       # [C, 3C]
    qkvb = np.ascontiguousarray(qkv_bias[:2 * C].reshape(16, P).T)  # [128, 16]
    vbias = np.ascontiguousarray(qkv_bias[2 * C:].reshape(1, C))
    wproj = np.ascontiguousarray(proj_w.T.astype(f))
    projb = np.ascontiguousarray(proj_b.reshape(KC, P).T.astype(f))
    wfc1 = np.ascontiguousarray((fc1_w * ln2_w[None, :]).T.astype(f))
    fc1b = np.ascontiguousarray((fc1_w @ ln2_b + fc1_b).reshape(MC_FC1, P).T.astype(f))
    ones = np.ones((1, P), f)
    return dict(wqkv=wqkv, qkvb=qkvb, vbias=vbias, wproj=wproj, projb=projb,
                wfc1=wfc1, fc1b=fc1b, ones=ones)


def _get_runner():
    if "runner" in _CACHE:
        return _CACHE["runner"]
    import jax
    from jax.sharding import Mesh, PartitionSpec
    from jax.experimental.shard_map import shard_map
    from concourse import bass2jax, mybir as _mybir

    nc = build()
    nc.compile()
    bass2jax.install_neuronx_cc_hook()
    partition_name = nc.partition_id_tensor.name if nc.partition_id_tensor else None
    in_names, out_names, out_avals, zero_outs = [], [], [], []
    for alloc in nc.m.functions[0].allocations:
        if not isinstance(alloc, _mybir.MemoryLocationSet):
            continue
        name = alloc.memorylocations[0].name
        if alloc.kind == "ExternalInput":
            if name != partition_name:
                in_names.append(name)
        elif alloc.kind == "ExternalOutput":
            shape = tuple(alloc.tensor_shape)
            dtype = _mybir.dt.np(alloc.dtype)
            out_names.append(name)
            out_avals.append(jax.core.ShapedArray(shape, dtype))
            zero_outs.append(np.zeros(shape, dtype))
    n_params = len(in_names)
    n_outs = len(out_avals)
    all_names = list(in_names) + list(out_names)
    if partition_name is not None:
        all_names.append(partition_name)

    def _body(*args):
        operands = list(args)
        if partition_name is not None:
            operands.append(bass2jax.partition_id_tensor())
        outs = bass2jax._bass_exec_p.bind(
            *operands,
            out_avals=tuple(out_avals),
            in_names=tuple(all_names),
            out_names=tuple(out_names),
            lowering_input_output_aliases=(),
            sim_require_finite=True,
            sim_require_nnan=True,
            nc=nc,
        )
        return tuple(outs)

    n_cores = 8
    devices = jax.devices()[:n_cores]
    mesh = Mesh(np.asarray(devices), ("core",))
    in_specs = (PartitionSpec("core"),) * (n_params + n_outs)
    out_specs = (PartitionSpec("core"),) * n_outs
    sharded = jax.jit(
        shard_map(_body, mesh=mesh, in_specs=in_specs, out_specs=out_specs, check_rep=False),
        donate_argnums=tuple(range(n_params, n_params + n_outs)),
        keep_unused=True,
    )

    def run(in_maps):
        concat_in = [np.concatenate([np.asarray(in_maps[c][nm]) for c in range(n_cores)], axis=0)
                     for nm in in_names]
        concat_zeros = [np.zeros((n_cores * z.shape[0], *z.shape[1:]), z.dtype) for z in zero_outs]
        out_arrs = sharded(*concat_in, *concat_zeros)
        return [
            {nm: np.asarray(out_arrs[i]).reshape(n_cores, *out_avals[i].shape)[c]
             for i, nm in enumerate(out_names)}
            for c in range(n_cores)
        ]

    _CACHE["runner"] = run
    return run


def make_in_maps(x, indices, ln1_w, ln1_b, qkv_w, proj_w, proj_b,
                 ln2_w, ln2_b, fc1_w, fc1_b, fc2_w, fc2_b, exp_w, exp_b):
    x = np.asarray(x, np.float32)
    indices = np.asarray(indices)
    shared = _prep_shared(
        np.asarray(ln1_w, np.float32), np.asarray(ln1_b, np.float32),
        np.asarray(qkv_w, np.float32), np.asarray(proj_w, np.float32),
        np.asarray(proj_b, np.float32), np.asarray(ln2_w, np.float32),
        np.asarray(ln2_b, np.float32), np.asarray(fc1_w, np.float32),
        np.asarray(fc1_b, np.float32), np.asarray(fc2_w, np.float32),
        np.asarray(fc2_b, np.float32))
    fc2_w = np.asarray(fc2_w, np.float32)
    fc2_b = np.asarray(fc2_b, np.float32)
    exp_w = np.asarray(exp_w, np.float32)
    exp_b = np.asarray(exp_b, np.float32)

    in_maps = []
    for core in range(8):
        b, half = core // 2, core % 2
        # roll tokens so this core's half occupies columns [0, T); attention
        # is permutation-invariant over keys so K/V order doesn't matter
        xb = x[b]
        if half:
            xb = np.concatenate([xb[T:], xb[:T]], axis=0)
        xt = np.ascontiguousarray(xb.T)
        e = int(indices[b])
        w2 = np.ascontiguousarray(np.concatenate([fc2_w, exp_w[e]], axis=0).T)
        b2 = np.ascontiguousarray(
            np.concatenate([fc2_b, exp_b[e]]).reshape(KC, P).T)
        m = dict(shared)
        m["xt"] = xt
        m["w2"] = w2
        m["b2"] = b2
        in_maps.append(m)
    return in_maps


def assemble_output(outs):
    y = np.empty((B, N, C), np.float32)
    for core in range(8):
        b, half = core // 2, core % 2
        y[b, half * T:(half + 1) * T] = outs[core]["out"].T
    return y


def kernel(x, indices, ln1_w, ln1_b, qkv_w, proj_w, proj_b,
           ln2_w, ln2_b, fc1_w, fc1_b, fc2_w, fc2_b, exp_w, exp_b):
    in_maps = make_in_maps(x, indices, ln1_w, ln1_b, qkv_w, proj_w, proj_b,
                           ln2_w, ln2_b, fc1_w, fc1_b, fc2_w, fc2_b, exp_w, exp_b)
    run = _get_runner()
    outs = run(in_maps)
    return assemble_output(outs)


# revision 16
# speedup vs baseline: 1.1868x; 1.0101x over previous
"""Trainium2 Bass kernel for nn_Block_6433861009667 (transformer block w/ MoE MLP).

Sharding: 8 cores = (batch sample b = core//2) x (token half = core%2).
Each core computes its 512 tokens fully locally (K/V computed for the whole
sample on both cores of a pair -> no collectives at all).

Device layout: feature-major activations [C(partitions), T(free)], fp32r
matmuls (full PE rate, ~1.4e-4 matmul error), softmax denominators computed
on the PE via packed M=1 ones-matmuls, LayerNorm scale/shift folded into the
following matmul weights on the host.
"""
import sys

if "/opt/trn_rl_repo" not in sys.path:
    sys.path.insert(0, "/opt/trn_rl_repo")

import numpy as np

import concourse.bass as bass
import concourse.tile as tile
from concourse import bacc, mybir
from concourse.bass import ts

B, N, C = 4, 1024, 1024
H, HD = 16, 64
HID = 4 * C
PF, SHARED = 256, C - 256
EPS = 1e-5
T = 512            # tokens per core
P = 128
KC = C // P        # 8 feature chunks
MC_FC1 = HID // P  # 32
F32R = mybir.dt.float32r
F32 = mybir.dt.float32
AF = mybir.ActivationFunctionType


def build():
    nc = bacc.Bacc(trn_type="TRN2")

    # ---- DRAM I/O (per core) ----
    xt_d = nc.dram_tensor("xt", [C, N], F32R, kind="ExternalInput")          # x[b].T (tokens rolled so own half = cols 0:T)
    wqkv_d = nc.dram_tensor("wqkv", [C, 3 * C], F32R, kind="ExternalInput")  # folded (qkv_w*ln1w).T, Q cols pre-scaled
    qkvb_d = nc.dram_tensor("qkvb", [P, 16], F32R, kind="ExternalInput")     # Q/K bias per m-chunk column
    vbias_d = nc.dram_tensor("vbias", [1, C], F32R, kind="ExternalInput")    # V bias row
    wproj_d = nc.dram_tensor("wproj", [C, C], F32R, kind="ExternalInput")    # proj_w.T
    projb_d = nc.dram_tensor("projb", [P, KC], F32R, kind="ExternalInput")
    wfc1_d = nc.dram_tensor("wfc1", [C, HID], F32R, kind="ExternalInput")    # (fc1_w*ln2w).T
    fc1b_d = nc.dram_tensor("fc1b", [P, MC_FC1], F32R, kind="ExternalInput")
    w2_d = nc.dram_tensor("w2", [HID, C], F32R, kind="ExternalInput")        # concat([fc2_w, exp_w[b]]).T
    b2_d = nc.dram_tensor("b2", [P, KC], F32R, kind="ExternalInput")
    ones_d = nc.dram_tensor("ones", [1, P], F32R, kind="ExternalInput")
    out_d = nc.dram_tensor("out", [C, T], F32, kind="ExternalOutput")        # y[b, half].T

    with tile.TileContext(nc, pool_alloc_mode="queue") as tc:
        # ---- whole-kernel pools ----
        root = tc.alloc_tile_pool(name="root", bufs=1)
        tmp = tc.alloc_tile_pool(name="tmp", bufs=2)
        small = tc.alloc_tile_pool(name="small", bufs=1)
        wpool = tc.alloc_tile_pool(name="w", bufs=4)
        # PSUM budget: 3 + 2 + 2 = 7 banks of 8
        mm_ps = tc.alloc_tile_pool(name="mmps", bufs=3, space="PSUM")   # dense mm + attn S
        b2_ps = tc.alloc_tile_pool(name="b2ps", bufs=2, space="PSUM")   # attn O + LN stats
        b3_ps = tc.alloc_tile_pool(name="b3ps", bufs=2, space="PSUM")   # attn D + replicate

        # ---- constants / biases ----
        ones_sq = root.tile([P, P], F32R, tag="ones_sq")       # all-ones
        nc.sync.dma_start(ones_sq[:], ones_d.ap()[0, :].partition_broadcast(P))
        ones_col = root.tile([P, 1], F32R, tag="ones_col")
        nc.sync.dma_start(ones_col[:], ones_d.ap()[0, 0:1].partition_broadcast(P))
        qkvb_s = root.tile([P, 16], F32R, tag="qkvb")
        nc.sync.dma_start(qkvb_s[:], qkvb_d.ap())
        vbias_s = root.tile([1, C], F32R, tag="vbias")
        nc.sync.dma_start(vbias_s[:], vbias_d.ap())
        projb_s = root.tile([P, KC], F32R, tag="projb")
        nc.sync.dma_start(projb_s[:], projb_d.ap())
        fc1b_s = root.tile([P, MC_FC1], F32R, tag="fc1b")
        nc.sync.dma_start(fc1b_s[:], fc1b_d.ap())
        b2_s = root.tile([P, KC], F32R, tag="b2")
        nc.sync.dma_start(b2_s[:], b2_d.ap())
        eps_t = root.tile([1, 1], F32, tag="eps")
        nc.vector.memset(eps_t[:], EPS)

        # =============== helper: feature-major LayerNorm ===============
        def layernorm_fm(src, dst, tokens, aB, cB):
            """src/dst: [P, KC, tokens] f32r tiles. dst = (src - mu)*rstd per token."""
            halves = tokens // T
            negmu = small.tile([1, N], F32R, tag="lnm", name=f"negmu{tokens}")
            msq = small.tile([1, N], F32R, tag="lns", name=f"msq{tokens}")
            crow = small.tile([1, N], F32R, tag="lnc", name=f"crow{tokens}")
            for h in range(halves):
                sum_ps = b2_ps.tile([1, T], F32, tag="bank2", name=f"sum_ps{tokens}_{h}")
                sq_ps = b2_ps.tile([1, T], F32, tag="bank2", name=f"sq_ps{tokens}_{h}")
                for kc in range(KC):
                    xsq = tmp.tile([P, T], F32R, tag="xsq", name=f"xsq{tokens}_{h}_{kc}")
                    nc.vector.tensor_mul(xsq[:], src[:, kc, ts(h, T)], src[:, kc, ts(h, T)])
                    nc.tensor.matmul(sum_ps[:], ones_col[:], src[:, kc, ts(h, T)],
                                     start=(kc == 0), stop=(kc == KC - 1))
                    nc.tensor.matmul(sq_ps[:], ones_col[:], xsq[:],
                                     start=(kc == 0), stop=(kc == KC - 1))
                nc.scalar.mul(negmu[:, ts(h, T)], sum_ps[:], -1.0 / C)
                nc.scalar.mul(msq[:, ts(h, T)], sq_ps[:], 1.0 / C)
            tk = slice(0, tokens)
            nc.vector.tensor_mul(crow[:, tk], negmu[:, tk], negmu[:, tk])
            nc.vector.tensor_sub(msq[:, tk], msq[:, tk], crow[:, tk])
            # rstd = exp(-0.5*ln(var+eps))  (stays inside the exp/ln ACT table set)
            nc.scalar.activation(msq[:, tk], msq[:, tk], AF.Ln, bias=eps_t[0:1, :])
            nc.scalar.activation(msq[:, tk], msq[:, tk], AF.Exp, scale=-0.5)
            nc.vector.tensor_mul(crow[:, tk], negmu[:, tk], msq[:, tk])
            for h in range(halves):
                ra = b3_ps.tile([P, T], F32, tag="bank3", name=f"ra{tokens}_{h}")
                nc.tensor.matmul(ra[:], ones_sq[0:1, :], msq[:, ts(h, T)], start=True, stop=True)
                nc.scalar.copy(aB[:, ts(h, T)], ra[:])
                rc = b3_ps.tile([P, T], F32, tag="bank3", name=f"rc{tokens}_{h}")
                nc.tensor.matmul(rc[:], ones_sq[0:1, :], crow[:, ts(h, T)], start=True, stop=True)
                nc.scalar.copy(cB[:, ts(h, T)], rc[:])
            for kc in range(KC):
                nc.vector.tensor_mul(dst[:, kc, :], src[:, kc, :], aB[:])
                nc.vector.tensor_add(dst[:, kc, :], dst[:, kc, :], cB[:])

        # =============== LN1 ===============
        pLN1 = tc.alloc_tile_pool(name="pLN1", bufs=1)    # ln1 (until V done)
        ln1 = pLN1.tile([P, KC, N], F32R, tag="ln1")
        pWV = tc.alloc_tile_pool(name="pWV", bufs=1)      # wv (until V done)
        wv_s = pWV.tile([P, KC, C], F32R, tag="wv")
        nc.sync.dma_start(wv_s[:], wqkv_d.ap()[:, 2 * C:3 * C].rearrange("(kc p) m -> p kc m", p=P))

        pXT = tc.alloc_tile_pool(name="pXT", bufs=1)      # xt + LN1 broadcast rows
        xt_s = pXT.tile([P, KC, N], F32R, tag="xt")
        nc.sync.dma_start(xt_s[:], xt_d.ap().rearrange("(kc p) t -> p kc t", p=P))
        aB1 = pXT.tile([P, N], F32R, tag="aB1")
        cB1 = pXT.tile([P, N], F32R, tag="cB1")
        layernorm_fm(xt_s, ln1, N, aB1, cB1)
        pXT.release()

        # =============== QKV ===============
        pQ = tc.alloc_tile_pool(name="pQ", bufs=1, side="right")
        pK = tc.alloc_tile_pool(name="pK", bufs=1, side="right")
        pV = tc.alloc_tile_pool(name="pV", bufs=1, side="right")
        q_s = pQ.tile([P, KC, T], F32R, tag="q")
        k_s = pK.tile([P, KC, N], F32R, tag="k")
        vp = pV.tile([P, KC, H, HD + 1], F32R, tag="vp")

        # Q: out[m, t] for this core's (pre-rolled) token half = cols 0:T
        for m in range(KC):
            wt = wpool.tile([P, KC, P], F32R, tag="w128", name=f"wq{m}")
            nc.sync.dma_start(wt[:], wqkv_d.ap()[:, ts(m, P)].rearrange("(kc p) m -> p kc m", p=P))
            ps = mm_ps.tile([P, T], F32, tag="mm", name=f"psq{m}")
            for kc in range(KC):
                nc.tensor.matmul(ps[:], wt[:, kc, :], ln1[:, kc, 0:T],
                                 start=(kc == 0), stop=(kc == KC - 1))
            nc.scalar.activation(q_s[:, m, :], ps[:], AF.Identity, bias=qkvb_s[:, m:m + 1])
        # K: out[m, all N tokens]
        for m in range(KC):
            wt = wpool.tile([P, KC, P], F32R, tag="w128", name=f"wk{m}")
            nc.sync.dma_start(wt[:], wqkv_d.ap()[:, C + m * P:C + (m + 1) * P].rearrange("(kc p) m -> p kc m", p=P))
            for h in range(2):
                ps = mm_ps.tile([P, T], F32, tag="mm", name=f"psk{m}_{h}")
                for kc in range(KC):
                    nc.tensor.matmul(ps[:], wt[:, kc, :], ln1[:, kc, ts(h, T)],
                                     start=(kc == 0), stop=(kc == KC - 1))
                nc.scalar.activation(k_s[:, m, ts(h, T)], ps[:], AF.Identity,
                                     bias=qkvb_s[:, 8 + m:9 + m])
        # V (token-major): out[j, d] ; lhsT = ln1 chunk (j slice), rhs = wv
        for jc in range(KC):
            for dh in range(2):
                ps = mm_ps.tile([P, T], F32, tag="mm", name=f"psv{jc}_{dh}")
                for kc in range(KC):
                    nc.tensor.matmul(ps[:], ln1[:, kc, ts(jc, P)], wv_s[:, kc, ts(dh, T)],
                                     start=(kc == 0), stop=False)
                nc.tensor.matmul(ps[:], ones_sq[0:1, :], vbias_s[:, ts(dh, T)],
                                 start=False, stop=True)
                nc.vector.tensor_copy(vp[:, jc, dh * 8:(dh + 1) * 8, 0:HD],
                                      ps[:].rearrange("p (g d) -> p g d", g=8))
        for jc in range(KC):
            nc.scalar.copy(vp[:, jc, :, HD:HD + 1], ones_sq[:, 0:H, None])
        pWV.release()
        pLN1.release()

        # =============== Attention (head pairs) ===============
        pX1 = tc.alloc_tile_pool(name="pX1", bufs=1)      # x1 (until end; below pATT on left stack)
        pATT = tc.alloc_tile_pool(name="pATT", bufs=1)    # attn (until proj done)
        attn = pATT.tile([P, KC, T], F32R, tag="attn")
        pPH = tc.alloc_tile_pool(name="pPH", bufs=16, side="right")      # per-(head,jc) P^T tiles

        def s_mms(c):
            phs = [[None] * KC, [None] * KC]
            for jc in range(KC):
                for g in range(2):
                    b0 = g * 64
                    ps = mm_ps.tile([P, T], F32, tag="mm", name=f"s{c}_{g}_{jc}")
                    nc.tensor.matmul(ps[:], k_s[b0:b0 + 64, c, ts(jc, P)],
                                     q_s[b0:b0 + 64, c, :], start=True, stop=True)
                    ph = pPH.tile([P, T], F32R, tag="ph", name=f"ph{c}_{g}_{jc}")
                    nc.scalar.activation(ph[:], ps[:], AF.Exp)
                    phs[g][jc] = ph
            return phs

        def pv_mms(c, phs):
            for g in range(2):
                h = 2 * c + g
                pso = b2_ps.tile([HD + 1, T], F32, tag="bank2", name=f"pso{c}_{g}")
                for jc in range(KC):
                    nc.tensor.matmul(pso[:], vp[:, jc, h, :], phs[g][jc][:],
                                     start=(jc == 0), stop=(jc == KC - 1))
                rcp = tmp.tile([HD + 1, T], F32R, tag="rcp", name=f"rcp{c}_{g}")
                # 1/D = exp(-ln(D)) on ACT: cheap, same table set as the attention exp
                nc.scalar.activation(rcp[HD:HD + 1, :], pso[HD:HD + 1, :], AF.Ln)
                nc.scalar.activation(rcp[HD:HD + 1, :], rcp[HD:HD + 1, :], AF.Exp, scale=-1.0)
                psr = b3_ps.tile([HD, T], F32, tag="bank3", name=f"psr{c}_{g}")
                nc.tensor.matmul(psr[:], ones_sq[HD:HD + 1, 0:HD], rcp[HD:HD + 1, :],
                                 start=True, stop=True)
                if g == 0:
                    nc.vector.tensor_copy(attn[0:HD, c, :], pso[0:HD, :])
                    nc.vector.tensor_mul(attn[0:HD, c, :], attn[0:HD, c, :], psr[:])
                else:
                    ot = tmp.tile([HD, T], F32R, tag="ot", name=f"ot{c}")
                    nc.vector.tensor_copy(ot[:], pso[0:HD, :])
                    nc.vector.tensor_mul(ot[:], ot[:], psr[:])
                    nc.sync.dma_start(attn[HD:P, c, :], ot[:])

        prev = s_mms(0)
        for c in range(KC):
            cur = prev
            if c + 1 < KC:
                prev = s_mms(c + 1)
            pv_mms(c, cur)
        pPH.release()
        pV.release()
        pK.release()
        pQ.release()

        # =============== proj + residual ===============
        x1 = pX1.tile([P, KC, T], F32R, tag="x1")
        for m in range(KC):
            wt = wpool.tile([P, KC, P], F32R, tag="w128", name=f"wp{m}")
            nc.sync.dma_start(wt[:], wproj_d.ap()[:, ts(m, P)].rearrange("(kc p) m -> p kc m", p=P))
            xh = tmp.tile([P, T], F32R, tag="xh", name=f"xh{m}")
            nc.sync.dma_start(xh[:], xt_d.ap()[ts(m, P), 0:T])
            ps = mm_ps.tile([P, T], F32, tag="mm", name=f"psp{m}")
            for kc in range(KC):
                nc.tensor.matmul(ps[:], wt[:, kc, :], attn[:, kc, :],
                                 start=(kc == 0), stop=(kc == KC - 1))
            po = tmp.tile([P, T], F32R, tag="po", name=f"po{m}")
            nc.scalar.activation(po[:], ps[:], AF.Identity, bias=projb_s[:, m:m + 1])
            nc.vector.tensor_add(x1[:, m, :], po[:], xh[:])
        pATT.release()

        # =============== LN2 ===============
        pLN2 = tc.alloc_tile_pool(name="pLN2", bufs=1, side="right")
        ln2t = pLN2.tile([P, KC, T], F32R, tag="ln2t")
        aB2 = pLN2.tile([P, T], F32R, tag="aB2")
        cB2 = pLN2.tile([P, T], F32R, tag="cB2")
        layernorm_fm(x1, ln2t, T, aB2, cB2)

        # =============== FC1 + gelu ===============
        pH = tc.alloc_tile_pool(name="pH", bufs=1)
        hbuf = pH.tile([P, MC_FC1, T], F32R, tag="h")
        for m in range(MC_FC1):
            wt = wpool.tile([P, KC, P], F32R, tag="w128", name=f"wf{m}")
            nc.sync.dma_start(wt[:], wfc1_d.ap()[:, ts(m, P)].rearrange("(kc p) m -> p kc m", p=P))
            ps = mm_ps.tile([P, T], F32, tag="mm", name=f"psf{m}")
            for kc in range(KC):
                nc.tensor.matmul(ps[:], wt[:, kc, :], ln2t[:, kc, :],
                                 start=(kc == 0), stop=(kc == KC - 1))
            nc.scalar.activation(hbuf[:, m, :], ps[:], AF.Gelu, bias=fc1b_s[:, m:m + 1])
        pLN2.release()

        # =============== FC2 (shared+expert fused) + residual + out ===============
        pW2 = tc.alloc_tile_pool(name="pW2", bufs=3, side="right")
        pY = tc.alloc_tile_pool(name="pY", bufs=1, side="right")
        ybuf = pY.tile([P, KC, T], F32, tag="y")
        for m in range(KC):
            wt = pW2.tile([P, MC_FC1, P], F32R, tag="w2t", name=f"w2_{m}")
            nc.sync.dma_start(wt[:], w2_d.ap()[:, ts(m, P)].rearrange("(kc p) m -> p kc m", p=P))
            ps = mm_ps.tile([P, T], F32, tag="mm", name=f"ps2_{m}")
            for kc in range(MC_FC1):
                nc.tensor.matmul(ps[:], wt[:, kc, :], hbuf[:, kc, :],
                                 start=(kc == 0), stop=(kc == MC_FC1 - 1))
            po = tmp.tile([P, T], F32R, tag="po", name=f"po2_{m}")
            nc.scalar.activation(po[:], ps[:], AF.Identity, bias=b2_s[:, m:m + 1])
            nc.vector.tensor_add(ybuf[:, m, :], po[:], x1[:, m, :])
            nc.sync.dma_start(out_d.ap()[ts(m, P), :], ybuf[:, m, :])
        pH.release()
        pY.release()
        pW2.release()
        pX1.release()
        wpool.release()
        small.release()
        tmp.release()
        root.release()
        b3_ps.release()
        b2_ps.release()
        mm_ps.release()

    return nc


# ===================== host side =====================

_CACHE = {}


def _prep_shared(ln1_w, ln1_b, qkv_w, proj_w, proj_b, ln2_w, ln2_b,
                 fc1_w, fc1_b, fc2_w, fc2_b):
    f = np.float32
    scale = np.float32(HD ** -0.5)
    wq = (qkv_w * ln1_w[None, :]).astype(f)
    qkv_bias = (qkv_w @ ln1_b).astype(f)
    wq[:C] *= scale
    qkv_bias[:C] *= scale
    wqkv = np.ascontiguousarray(wq.T)                       # [C, 3C]
    qkvb = np.ascontiguousarray(qkv_bias[:2 * C].reshape(16, P).T)  # [128, 16]
    vbias = np.ascontiguousarray(qkv_bias[2 * C:].reshape(1, C))
    wproj = np.ascontiguousarray(proj_w.T.astype(f))
    projb = np.ascontiguousarray(proj_b.reshape(KC, P).T.astype(f))
    wfc1 = np.ascontiguousarray((fc1_w * ln2_w[None, :]).T.astype(f))
    fc1b = np.ascontiguousarray((fc1_w @ ln2_b + fc1_b).reshape(MC_FC1, P).T.astype(f))
    ones = np.ones((1, P), f)
    return dict(wqkv=wqkv, qkvb=qkvb, vbias=vbias, wproj=wproj, projb=projb,
                wfc1=wfc1, fc1b=fc1b, ones=ones)


def _get_runner():
    if "runner" in _CACHE:
        return _CACHE["runner"]
    import jax
    from jax.sharding import Mesh, PartitionSpec
    from jax.experimental.shard_map import shard_map
    from concourse import bass2jax, mybir as _mybir

    nc = build()
    nc.compile()
    bass2jax.install_neuronx_cc_hook()
    partition_name = nc.partition_id_tensor.name if nc.partition_id_tensor else None
    in_names, out_names, out_avals, zero_outs = [], [], [], []
    for alloc in nc.m.functions[0].allocations:
        if not isinstance(alloc, _mybir.MemoryLocationSet):
            continue
        name = alloc.memorylocations[0].name
        if alloc.kind == "ExternalInput":
            if name != partition_name:
                in_names.append(name)
        elif alloc.kind == "ExternalOutput":
            shape = tuple(alloc.tensor_shape)
            dtype = _mybir.dt.np(alloc.dtype)
            out_names.append(name)
            out_avals.append(jax.core.ShapedArray(shape, dtype))
            zero_outs.append(np.zeros(shape, dtype))
    n_params = len(in_names)
    n_outs = len(out_avals)
    all_names = list(in_names) + list(out_names)
    if partition_name is not None:
        all_names.append(partition_name)

    def _body(*args):
        operands = list(args)
        if partition_name is not None:
            operands.append(bass2jax.partition_id_tensor())
        outs = bass2jax._bass_exec_p.bind(
            *operands,
            out_avals=tuple(out_avals),
            in_names=tuple(all_names),
            out_names=tuple(out_names),
            lowering_input_output_aliases=(),
            sim_require_finite=True,
            sim_require_nnan=True,
            nc=nc,
        )
        return tuple(outs)

    n_cores = 8
    devices = jax.devices()[:n_cores]
    mesh = Mesh(np.asarray(devices), ("core",))
    in_specs = (PartitionSpec("core"),) * (n_params + n_outs)
    out_specs = (PartitionSpec("core"),) * n_outs
    sharded = jax.jit(
        shard_map(_body, mesh=mesh, in_specs=in_specs, out_specs=out_specs, check_rep=False),
        donate_argnums=tuple(range(n_params, n_params + n_outs)),
        keep_unused=True,
    )

    def run(in_maps):
        concat_in = [np.concatenate([np.asarray(in_maps[c][nm]) for c in range(n_cores)], axis=0)
                     for nm in in_names]
        concat_zeros = [np.zeros((n_cores * z.shape[0], *z.shape[1:]), z.dtype) for z in zero_outs]
        out_arrs = sharded(*concat_in, *concat_zeros)
        return [
            {nm: np.asarray(out_arrs[i]).reshape(n_cores, *out_avals[i].shape)[c]
             for i, nm in enumerate(out_names)}
            for c in range(n_cores)
        ]

    _CACHE["runner"] = run
    return run


def make_in_maps(x, indices, ln1_w, ln1_b, qkv_w, proj_w, proj_b,
                 ln2_w, ln2_b, fc1_w, fc1_b, fc2_w, fc2_b, exp_w, exp_b):
    x = np.asarray(x, np.float32)
    indices = np.asarray(indices)
    shared = _prep_shared(
        np.asarray(ln1_w, np.float32), np.asarray(ln1_b, np.float32),
        np.asarray(qkv_w, np.float32), np.asarray(proj_w, np.float32),
        np.asarray(proj_b, np.float32), np.asarray(ln2_w, np.float32),
        np.asarray(ln2_b, np.float32), np.asarray(fc1_w, np.float32),
        np.asarray(fc1_b, np.float32), np.asarray(fc2_w, np.float32),
        np.asarray(fc2_b, np.float32))
    fc2_w = np.asarray(fc2_w, np.float32)
    fc2_b = np.asarray(fc2_b, np.float32)
    exp_w = np.asarray(exp_w, np.float32)
    exp_b = np.asarray(exp_b, np.float32)

    in_maps = []
    for core in range(8):
        b, half = core // 2, core % 2
        # roll tokens so this core's half occupies columns [0, T); attention
        # is permutation-invariant over keys so K/V order doesn't matter
        xb = x[b]
        if half:
            xb = np.concatenate([xb[T:], xb[:T]], axis=0)
        xt = np.ascontiguousarray(xb.T)
        e = int(indices[b])
        w2 = np.ascontiguousarray(np.concatenate([fc2_w, exp_w[e]], axis=0).T)
        b2 = np.ascontiguousarray(
            np.concatenate([fc2_b, exp_b[e]]).reshape(KC, P).T)
        m = dict(shared)
        m["xt"] = xt
        m["w2"] = w2
        m["b2"] = b2
        in_maps.append(m)
    return in_maps


def assemble_output(outs):
    y = np.empty((B, N, C), np.float32)
    for core in range(8):
        b, half = core // 2, core % 2
        y[b, half * T:(half + 1) * T] = outs[core]["out"].T
    return y


def kernel(x, indices, ln1_w, ln1_b, qkv_w, proj_w, proj_b,
           ln2_w, ln2_b, fc1_w, fc1_b, fc2_w, fc2_b, exp_w, exp_b):
    in_maps = make_in_maps(x, indices, ln1_w, ln1_b, qkv_w, proj_w, proj_b,
                           ln2_w, ln2_b, fc1_w, fc1_b, fc2_w, fc2_b, exp_w, exp_b)
    run = _get_runner()
    outs = run(in_maps)
    return assemble_output(outs)


# revision 21
# speedup vs baseline: 1.2157x; 1.0243x over previous
"""Trainium2 Bass kernel for nn_Block_6433861009667 (transformer block w/ MoE MLP).

Sharding: 8 cores = (batch sample b = core//2) x (token half = core%2).
Each core computes its 512 tokens fully locally (K/V computed for the whole
sample on both cores of a pair -> no collectives at all).

Device layout: feature-major activations [C(partitions), T(free)], fp32r
matmuls (full PE rate, ~1.4e-4 matmul error), softmax denominators computed
on the PE via packed M=1 ones-matmuls, LayerNorm scale/shift folded into the
following matmul weights on the host.
"""
import sys

if "/opt/trn_rl_repo" not in sys.path:
    sys.path.insert(0, "/opt/trn_rl_repo")

import numpy as np

import concourse.bass as bass
import concourse.tile as tile
from concourse import bacc, mybir
from concourse.bass import ts

B, N, C = 4, 1024, 1024
H, HD = 16, 64
HID = 4 * C
PF, SHARED = 256, C - 256
EPS = 1e-5
T = 512            # tokens per core
P = 128
KC = C // P        # 8 feature chunks
MC_FC1 = HID // P  # 32
F32R = mybir.dt.float32r
F32 = mybir.dt.float32
AF = mybir.ActivationFunctionType


def build():
    nc = bacc.Bacc(trn_type="TRN2")

    # ---- DRAM I/O (per core) ----
    xt_d = nc.dram_tensor("xt", [C, N], F32R, kind="ExternalInput")          # x[b].T (tokens rolled so own half = cols 0:T)
    wqkv_d = nc.dram_tensor("wqkv", [C, 3 * C], F32R, kind="ExternalInput")  # folded (qkv_w*ln1w).T, Q cols pre-scaled
    qkvb_d = nc.dram_tensor("qkvb", [P, 16], F32R, kind="ExternalInput")     # Q/K bias per m-chunk column
    vbias_d = nc.dram_tensor("vbias", [1, C], F32R, kind="ExternalInput")    # V bias row
    wproj_d = nc.dram_tensor("wproj", [C, C], F32R, kind="ExternalInput")    # proj_w.T
    projb_d = nc.dram_tensor("projb", [P, KC], F32R, kind="ExternalInput")
    wfc1_d = nc.dram_tensor("wfc1", [C, HID], F32R, kind="ExternalInput")    # (fc1_w*ln2w).T
    fc1b_d = nc.dram_tensor("fc1b", [P, MC_FC1], F32R, kind="ExternalInput")
    w2_d = nc.dram_tensor("w2", [HID, C], F32R, kind="ExternalInput")        # concat([fc2_w, exp_w[b]]).T
    b2_d = nc.dram_tensor("b2", [P, KC], F32R, kind="ExternalInput")
    ones_d = nc.dram_tensor("ones", [1, P], F32R, kind="ExternalInput")
    out_d = nc.dram_tensor("out", [C, T], F32, kind="ExternalOutput")        # y[b, half].T

    with tile.TileContext(nc, pool_alloc_mode="queue") as tc:
        # ---- whole-kernel pools ----
        root = tc.alloc_tile_pool(name="root", bufs=1)
        tmp = tc.alloc_tile_pool(name="tmp", bufs=2)
        small = tc.alloc_tile_pool(name="small", bufs=1)
        wpool = tc.alloc_tile_pool(name="w", bufs=4)
        # PSUM budget: 3 + 2 + 2 = 7 banks of 8
        mm_ps = tc.alloc_tile_pool(name="mmps", bufs=3, space="PSUM")   # dense mm + attn S
        b2_ps = tc.alloc_tile_pool(name="b2ps", bufs=2, space="PSUM")   # attn O + LN stats
        b3_ps = tc.alloc_tile_pool(name="b3ps", bufs=2, space="PSUM")   # attn D + replicate

        # ---- constants / biases ----
        ones_sq = root.tile([P, P], F32R, tag="ones_sq")       # all-ones
        nc.sync.dma_start(ones_sq[:], ones_d.ap()[0, :].partition_broadcast(P))
        ones_col = root.tile([P, 1], F32R, tag="ones_col")
        nc.sync.dma_start(ones_col[:], ones_d.ap()[0, 0:1].partition_broadcast(P))
        qkvb_s = root.tile([P, 16], F32R, tag="qkvb")
        nc.sync.dma_start(qkvb_s[:], qkvb_d.ap())
        vbias_s = root.tile([1, C], F32R, tag="vbias")
        nc.sync.dma_start(vbias_s[:], vbias_d.ap())
        projb_s = root.tile([P, KC], F32R, tag="projb")
        nc.sync.dma_start(projb_s[:], projb_d.ap())
        fc1b_s = root.tile([P, MC_FC1], F32R, tag="fc1b")
        nc.sync.dma_start(fc1b_s[:], fc1b_d.ap())
        b2_s = root.tile([P, KC], F32R, tag="b2")
        nc.sync.dma_start(b2_s[:], b2_d.ap())
        eps_t = root.tile([1, 1], F32, tag="eps")
        nc.vector.memset(eps_t[:], EPS)

        # =============== helper: feature-major LayerNorm ===============
        def layernorm_fm(src, dst, tokens, aB, cB):
            """src/dst: [P, KC, tokens] f32r tiles. dst = (src - mu)*rstd per token."""
            halves = tokens // T
            negmu = small.tile([1, N], F32R, tag="lnm", name=f"negmu{tokens}")
            msq = small.tile([1, N], F32R, tag="lns", name=f"msq{tokens}")
            crow = small.tile([1, N], F32R, tag="lnc", name=f"crow{tokens}")
            for h in range(halves):
                sum_ps = b2_ps.tile([1, T], F32, tag="bank2", name=f"sum_ps{tokens}_{h}")
                sq_ps = b2_ps.tile([1, T], F32, tag="bank2", name=f"sq_ps{tokens}_{h}")
                for kc in range(KC):
                    xsq = tmp.tile([P, T], F32R, tag="xsq", name=f"xsq{tokens}_{h}_{kc}")
                    nc.vector.tensor_mul(xsq[:], src[:, kc, ts(h, T)], src[:, kc, ts(h, T)])
                    nc.tensor.matmul(sum_ps[:], ones_col[:], src[:, kc, ts(h, T)],
                                     start=(kc == 0), stop=(kc == KC - 1))
                    nc.tensor.matmul(sq_ps[:], ones_col[:], xsq[:],
                                     start=(kc == 0), stop=(kc == KC - 1))
                nc.scalar.mul(negmu[:, ts(h, T)], sum_ps[:], -1.0 / C)
                nc.scalar.mul(msq[:, ts(h, T)], sq_ps[:], 1.0 / C)
            tk = slice(0, tokens)
            nc.vector.tensor_mul(crow[:, tk], negmu[:, tk], negmu[:, tk])
            nc.vector.tensor_sub(msq[:, tk], msq[:, tk], crow[:, tk])
            # rstd = exp(-0.5*ln(var+eps))  (stays inside the exp/ln ACT table set)
            nc.scalar.activation(msq[:, tk], msq[:, tk], AF.Ln, bias=eps_t[0:1, :])
            nc.scalar.activation(msq[:, tk], msq[:, tk], AF.Exp, scale=-0.5)
            nc.vector.tensor_mul(crow[:, tk], negmu[:, tk], msq[:, tk])
            for h in range(halves):
                ra = b3_ps.tile([P, T], F32, tag="bank3", name=f"ra{tokens}_{h}")
                nc.tensor.matmul(ra[:], ones_sq[0:1, :], msq[:, ts(h, T)], start=True, stop=True)
                nc.scalar.copy(aB[:, ts(h, T)], ra[:])
                rc = b3_ps.tile([P, T], F32, tag="bank3", name=f"rc{tokens}_{h}")
                nc.tensor.matmul(rc[:], ones_sq[0:1, :], crow[:, ts(h, T)], start=True, stop=True)
                nc.scalar.copy(cB[:, ts(h, T)], rc[:])
            for kc in range(KC):
                nc.vector.tensor_mul(dst[:, kc, :], src[:, kc, :], aB[:])
                nc.vector.tensor_add(dst[:, kc, :], dst[:, kc, :], cB[:])

        # =============== LN1 ===============
        pLN1 = tc.alloc_tile_pool(name="pLN1", bufs=1)    # ln1 (until V done)
        ln1 = pLN1.tile([P, KC, N], F32R, tag="ln1")
        pWV = tc.alloc_tile_pool(name="pWV", bufs=1)      # wv (until V done)
        wv_s = pWV.tile([P, KC, C], F32R, tag="wv")
        nc.sync.dma_start(wv_s[:], wqkv_d.ap()[:, 2 * C:3 * C].rearrange("(kc p) m -> p kc m", p=P))

        pXT = tc.alloc_tile_pool(name="pXT", bufs=1)      # xt + LN1 broadcast rows
        xt_s = pXT.tile([P, KC, N], F32R, tag="xt")
        nc.sync.dma_start(xt_s[:], xt_d.ap().rearrange("(kc p) t -> p kc t", p=P))
        aB1 = pXT.tile([P, N], F32R, tag="aB1")
        cB1 = pXT.tile([P, N], F32R, tag="cB1")
        layernorm_fm(xt_s, ln1, N, aB1, cB1)
        pXT.release()

        # =============== QKV ===============
        pQ = tc.alloc_tile_pool(name="pQ", bufs=1, side="right")
        pK = tc.alloc_tile_pool(name="pK", bufs=1, side="right")
        pV = tc.alloc_tile_pool(name="pV", bufs=1, side="right")
        q_s = pQ.tile([P, KC, T], F32R, tag="q")
        k_s = pK.tile([P, KC, N], F32R, tag="k")
        vp = pV.tile([P, KC, H, HD + 1], F32R, tag="vp")

        # Q: out[m, t] for this core's (pre-rolled) token half = cols 0:T
        for m in range(KC):
            wt = wpool.tile([P, KC, P], F32R, tag="w128", name=f"wq{m}")
            nc.sync.dma_start(wt[:], wqkv_d.ap()[:, ts(m, P)].rearrange("(kc p) m -> p kc m", p=P))
            ps = mm_ps.tile([P, T], F32, tag="mm", name=f"psq{m}")
            for kc in range(KC):
                nc.tensor.matmul(ps[:], wt[:, kc, :], ln1[:, kc, 0:T],
                                 start=(kc == 0), stop=(kc == KC - 1))
            nc.scalar.activation(q_s[:, m, :], ps[:], AF.Identity, bias=qkvb_s[:, m:m + 1])
        # K: out[m, all N tokens]
        for m in range(KC):
            wt = wpool.tile([P, KC, P], F32R, tag="w128", name=f"wk{m}")
            nc.sync.dma_start(wt[:], wqkv_d.ap()[:, C + m * P:C + (m + 1) * P].rearrange("(kc p) m -> p kc m", p=P))
            for h in range(2):
                ps = mm_ps.tile([P, T], F32, tag="mm", name=f"psk{m}_{h}")
                for kc in range(KC):
                    nc.tensor.matmul(ps[:], wt[:, kc, :], ln1[:, kc, ts(h, T)],
                                     start=(kc == 0), stop=(kc == KC - 1))
                nc.scalar.activation(k_s[:, m, ts(h, T)], ps[:], AF.Identity,
                                     bias=qkvb_s[:, 8 + m:9 + m])
        # V (token-major): out[j, d] ; lhsT = ln1 chunk (j slice), rhs = wv
        for jc in range(KC):
            for dh in range(2):
                ps = mm_ps.tile([P, T], F32, tag="mm", name=f"psv{jc}_{dh}")
                for kc in range(KC):
                    nc.tensor.matmul(ps[:], ln1[:, kc, ts(jc, P)], wv_s[:, kc, ts(dh, T)],
                                     start=(kc == 0), stop=False)
                nc.tensor.matmul(ps[:], ones_sq[0:1, :], vbias_s[:, ts(dh, T)],
                                 start=False, stop=True)
                nc.vector.tensor_copy(vp[:, jc, dh * 8:(dh + 1) * 8, 0:HD],
                                      ps[:].rearrange("p (g d) -> p g d", g=8))
        for jc in range(KC):
            nc.scalar.copy(vp[:, jc, :, HD:HD + 1], ones_sq[:, 0:H, None])
        pWV.release()
        pLN1.release()

        # =============== Attention (head pairs) ===============
        pX1 = tc.alloc_tile_pool(name="pX1", bufs=1)      # x1 (until end; below pATT on left stack)
        pATT = tc.alloc_tile_pool(name="pATT", bufs=1)    # attn (until proj done)
        attn = pATT.tile([P, KC, T], F32R, tag="attn")
        pPH = tc.alloc_tile_pool(name="pPH", bufs=16, side="right")      # per-(head,jc) P^T tiles

        def s_mms(c):
            phs = [[None] * KC, [None] * KC]
            for jc in range(KC):
                for g in range(2):
                    b0 = g * 64
                    ps = mm_ps.tile([P, T], F32, tag="mm", name=f"s{c}_{g}_{jc}")
                    nc.tensor.matmul(ps[:], k_s[b0:b0 + 64, c, ts(jc, P)],
                                     q_s[b0:b0 + 64, c, :], start=True, stop=True)
                    ph = pPH.tile([P, T], F32R, tag="ph", name=f"ph{c}_{g}_{jc}")
                    nc.scalar.activation(ph[:], ps[:], AF.Exp)
                    phs[g][jc] = ph
            return phs

        def pv_mms(c, phs):
            for g in range(2):
                h = 2 * c + g
                pso = b2_ps.tile([HD + 1, T], F32, tag="bank2", name=f"pso{c}_{g}")
                for jc in range(KC):
                    nc.tensor.matmul(pso[:], vp[:, jc, h, :], phs[g][jc][:],
                                     start=(jc == 0), stop=(jc == KC - 1))
                # reciprocal of D with tokens spread over partitions: DVE recip
                # cost scales with free size, so [128,4] makes it ~free; the two
                # small DMAs do the cross-partition reshape off the engines.
                rcpS = tmp.tile([HD + 1, T], F32R, tag="rcp", name=f"rcpS{c}_{g}")
                nc.vector.tensor_copy(rcpS[HD:HD + 1, :], pso[HD:HD + 1, :])
                rcpT = small.tile([P, T // P], F32R, tag="rcpT", name=f"rcpT{c}_{g}")
                nc.sync.dma_start(rcpT[:], rcpS[HD:HD + 1, :])
                with nc.allow_low_precision(reason="f32r bits are full fp32 here"):
                    nc.vector.reciprocal(rcpT[:], rcpT[:])
                rcp = tmp.tile([1, T], F32R, tag="rcp", name=f"rcp{c}_{g}")
                nc.sync.dma_start(rcp[:], rcpT[:])
                psr = b3_ps.tile([HD, T], F32, tag="bank3", name=f"psr{c}_{g}")
                nc.tensor.matmul(psr[:], ones_sq[0:1, 0:HD], rcp[0:1, :],
                                 start=True, stop=True)
                if g == 0:
                    nc.vector.tensor_copy(attn[0:HD, c, :], pso[0:HD, :])
                    nc.vector.tensor_mul(attn[0:HD, c, :], attn[0:HD, c, :], psr[:])
                else:
                    ot = tmp.tile([HD, T], F32R, tag="ot", name=f"ot{c}")
                    nc.vector.tensor_copy(ot[:], pso[0:HD, :])
                    nc.vector.tensor_mul(ot[:], ot[:], psr[:])
                    nc.sync.dma_start(attn[HD:P, c, :], ot[:])

        prev = s_mms(0)
        for c in range(KC):
            cur = prev
            if c + 1 < KC:
                prev = s_mms(c + 1)
            pv_mms(c, cur)
        pPH.release()
        pV.release()
        pK.release()
        pQ.release()

        # =============== proj + residual ===============
        x1 = pX1.tile([P, KC, T], F32R, tag="x1")
        for m in range(KC):
            wt = wpool.tile([P, KC, P], F32R, tag="w128", name=f"wp{m}")
            nc.sync.dma_start(wt[:], wproj_d.ap()[:, ts(m, P)].rearrange("(kc p) m -> p kc m", p=P))
            xh = tmp.tile([P, T], F32R, tag="xh", name=f"xh{m}")
            nc.sync.dma_start(xh[:], xt_d.ap()[ts(m, P), 0:T])
            ps = mm_ps.tile([P, T], F32, tag="mm", name=f"psp{m}")
            for kc in range(KC):
                nc.tensor.matmul(ps[:], wt[:, kc, :], attn[:, kc, :],
                                 start=(kc == 0), stop=(kc == KC - 1))
            po = tmp.tile([P, T], F32R, tag="po", name=f"po{m}")
            nc.scalar.activation(po[:], ps[:], AF.Identity, bias=projb_s[:, m:m + 1])
            nc.vector.tensor_add(x1[:, m, :], po[:], xh[:])
        pATT.release()

        # =============== LN2 ===============
        pLN2 = tc.alloc_tile_pool(name="pLN2", bufs=1, side="right")
        ln2t = pLN2.tile([P, KC, T], F32R, tag="ln2t")
        aB2 = pLN2.tile([P, T], F32R, tag="aB2")
        cB2 = pLN2.tile([P, T], F32R, tag="cB2")
        layernorm_fm(x1, ln2t, T, aB2, cB2)

        # =============== FC1 + gelu ===============
        pH = tc.alloc_tile_pool(name="pH", bufs=1)
        hbuf = pH.tile([P, MC_FC1, T], F32R, tag="h")
        for m in range(MC_FC1):
            wt = wpool.tile([P, KC, P], F32R, tag="w128", name=f"wf{m}")
            nc.sync.dma_start(wt[:], wfc1_d.ap()[:, ts(m, P)].rearrange("(kc p) m -> p kc m", p=P))
            ps = mm_ps.tile([P, T], F32, tag="mm", name=f"psf{m}")
            for kc in range(KC):
                nc.tensor.matmul(ps[:], wt[:, kc, :], ln2t[:, kc, :],
                                 start=(kc == 0), stop=(kc == KC - 1))
            nc.scalar.activation(hbuf[:, m, :], ps[:], AF.Gelu, bias=fc1b_s[:, m:m + 1])
        pLN2.release()

        # =============== FC2 (shared+expert fused) + residual + out ===============
        pW2 = tc.alloc_tile_pool(name="pW2", bufs=3, side="right")
        pY = tc.alloc_tile_pool(name="pY", bufs=1, side="right")
        ybuf = pY.tile([P, KC, T], F32, tag="y")
        for m in range(KC):
            wt = pW2.tile([P, MC_FC1, P], F32R, tag="w2t", name=f"w2_{m}")
            nc.sync.dma_start(wt[:], w2_d.ap()[:, ts(m, P)].rearrange("(kc p) m -> p kc m", p=P))
            ps = mm_ps.tile([P, T], F32, tag="mm", name=f"ps2_{m}")
            for kc in range(MC_FC1):
                nc.tensor.matmul(ps[:], wt[:, kc, :], hbuf[:, kc, :],
                                 start=(kc == 0), stop=(kc == MC_FC1 - 1))
            po = tmp.tile([P, T], F32R, tag="po", name=f"po2_{m}")
            nc.scalar.activation(po[:], ps[:], AF.Identity, bias=b2_s[:, m:m + 1])
            nc.vector.tensor_add(ybuf[:, m, :], po[:], x1[:, m, :])
            nc.sync.dma_start(out_d.ap()[ts(m, P), :], ybuf[:, m, :])
        pH.release()
        pY.release()
        pW2.release()
        pX1.release()
        wpool.release()
        small.release()
        tmp.release()
        root.release()
        b3_ps.release()
        b2_ps.release()
        mm_ps.release()

    return nc


# ===================== host side =====================

_CACHE = {}


def _prep_shared(ln1_w, ln1_b, qkv_w, proj_w, proj_b, ln2_w, ln2_b,
                 fc1_w, fc1_b, fc2_w, fc2_b):
    f = np.float32
    scale = np.float32(HD ** -0.5)
    wq = (qkv_w * ln1_w[None, :]).astype(f)
    qkv_bias = (qkv_w @ ln1_b).astype(f)
    wq[:C] *= scale
    qkv_bias[:C] *= scale
    wqkv = np.ascontiguousarray(wq.T)                       # [C, 3C]
    qkvb = np.ascontiguousarray(qkv_bias[:2 * C].reshape(16, P).T)  # [128, 16]
    vbias = np.ascontiguousarray(qkv_bias[2 * C:].reshape(1, C))
    wproj = np.ascontiguousarray(proj_w.T.astype(f))
    projb = np.ascontiguousarray(proj_b.reshape(KC, P).T.astype(f))
    wfc1 = np.ascontiguousarray((fc1_w * ln2_w[None, :]).T.astype(f))
    fc1b = np.ascontiguousarray((fc1_w @ ln2_b + fc1_b).reshape(MC_FC1, P).T.astype(f))
    ones = np.ones((1, P), f)
    return dict(wqkv=wqkv, qkvb=qkvb, vbias=vbias, wproj=wproj, projb=projb,
                wfc1=wfc1, fc1b=fc1b, ones=ones)


def _get_runner():
    if "runner" in _CACHE:
        return _CACHE["runner"]
    import jax
    from jax.sharding import Mesh, PartitionSpec
    from jax.experimental.shard_map import shard_map
    from concourse import bass2jax, mybir as _mybir

    nc = build()
    nc.compile()
    bass2jax.install_neuronx_cc_hook()
    partition_name = nc.partition_id_tensor.name if nc.partition_id_tensor else None
    in_names, out_names, out_avals, zero_outs = [], [], [], []
    for alloc in nc.m.functions[0].allocations:
        if not isinstance(alloc, _mybir.MemoryLocationSet):
            continue
        name = alloc.memorylocations[0].name
        if alloc.kind == "ExternalInput":
            if name != partition_name:
                in_names.append(name)
        elif alloc.kind == "ExternalOutput":
            shape = tuple(alloc.tensor_shape)
            dtype = _mybir.dt.np(alloc.dtype)
            out_names.append(name)
            out_avals.append(jax.core.ShapedArray(shape, dtype))
            zero_outs.append(np.zeros(shape, dtype))
    n_params = len(in_names)
    n_outs = len(out_avals)
    all_names = list(in_names) + list(out_names)
    if partition_name is not None:
        all_names.append(partition_name)

    def _body(*args):
        operands = list(args)
        if partition_name is not None:
            operands.append(bass2jax.partition_id_tensor())
        outs = bass2jax._bass_exec_p.bind(
            *operands,
            out_avals=tuple(out_avals),
            in_names=tuple(all_names),
            out_names=tuple(out_names),
            lowering_input_output_aliases=(),
            sim_require_finite=True,
            sim_require_nnan=True,
            nc=nc,
        )
        return tuple(outs)

    n_cores = 8
    devices = jax.devices()[:n_cores]
    mesh = Mesh(np.asarray(devices), ("core",))
    in_specs = (PartitionSpec("core"),) * (n_params + n_outs)
    out_specs = (PartitionSpec("core"),) * n_outs
    sharded = jax.jit(
        shard_map(_body, mesh=mesh, in_specs=in_specs, out_specs=out_specs, check_rep=False),
        donate_argnums=tuple(range(n_params, n_params + n_outs)),
        keep_unused=True,
    )

    def run(in_maps):
        concat_in = [np.concatenate([np.asarray(in_maps[c][nm]) for c in range(n_cores)], axis=0)
                     for nm in in_names]
        concat_zeros = [np.zeros((n_cores * z.shape[0], *z.shape[1:]), z.dtype) for z in zero_outs]
        out_arrs = sharded(*concat_in, *concat_zeros)
        return [
            {nm: np.asarray(out_arrs[i]).reshape(n_cores, *out_avals[i].shape)[c]
             for i, nm in enumerate(out_names)}
            for c in range(n_cores)
        ]

    _CACHE["runner"] = run
    return run


def make_in_maps(x, indices, ln1_w, ln1_b, qkv_w, proj_w, proj_b,
                 ln2_w, ln2_b, fc1_w, fc1_b, fc2_w, fc2_b, exp_w, exp_b):
    x = np.asarray(x, np.float32)
    indices = np.asarray(indices)
    shared = _prep_shared(
        np.asarray(ln1_w, np.float32), np.asarray(ln1_b, np.float32),
        np.asarray(qkv_w, np.float32), np.asarray(proj_w, np.float32),
        np.asarray(proj_b, np.float32), np.asarray(ln2_w, np.float32),
        np.asarray(ln2_b, np.float32), np.asarray(fc1_w, np.float32),
        np.asarray(fc1_b, np.float32), np.asarray(fc2_w, np.float32),
        np.asarray(fc2_b, np.float32))
    fc2_w = np.asarray(fc2_w, np.float32)
    fc2_b = np.asarray(fc2_b, np.float32)
    exp_w = np.asarray(exp_w, np.float32)
    exp_b = np.asarray(exp_b, np.float32)

    in_maps = []
    for core in range(8):
        b, half = core // 2, core % 2
        # roll tokens so this core's half occupies columns [0, T); attention
        # is permutation-invariant over keys so K/V order doesn't matter
        xb = x[b]
        if half:
            xb = np.concatenate([xb[T:], xb[:T]], axis=0)
        xt = np.ascontiguousarray(xb.T)
        e = int(indices[b])
        w2 = np.ascontiguousarray(np.concatenate([fc2_w, exp_w[e]], axis=0).T)
        b2 = np.ascontiguousarray(
            np.concatenate([fc2_b, exp_b[e]]).reshape(KC, P).T)
        m = dict(shared)
        m["xt"] = xt
        m["w2"] = w2
        m["b2"] = b2
        in_maps.append(m)
    return in_maps


def assemble_output(outs):
    y = np.empty((B, N, C), np.float32)
    for core in range(8):
        b, half = core // 2, core % 2
        y[b, half * T:(half + 1) * T] = outs[core]["out"].T
    return y


def kernel(x, indices, ln1_w, ln1_b, qkv_w, proj_w, proj_b,
           ln2_w, ln2_b, fc1_w, fc1_b, fc2_w, fc2_b, exp_w, exp_b):
    in_maps = make_in_maps(x, indices, ln1_w, ln1_b, qkv_w, proj_w, proj_b,
                           ln2_w, ln2_b, fc1_w, fc1_b, fc2_w, fc2_b, exp_w, exp_b)
    run = _get_runner()
    outs = run(in_maps)
    return assemble_output(outs)
